# revision 1
# baseline (speedup 1.0000x reference)
"""DGCNN (gnn_message_passing) Trainium2 Bass kernel.

Strategy (data-parallel over graphs, 8 graphs per NeuronCore):
  - Host builds, per graph, the dense normalized propagation operator
    S[d, s] = (mult(s->d) + I) / deg_out[d]  (512x512 f32), shipped
    transposed as 4 chunks of [128, 512].
  - On device, each GCN layer is two matmul stages per graph:
      lin  = h @ W          (node-major, PE, 4 chunks)
      msgT = lin^T-chunks stationary x S^T chunks -> [128f, 512d] PSUM
      h'   = tanh(msgT)     (ACT, feature-major back to SBUF)
  - Sort-pooling (top-64 by last channel, stable ties) is computed exactly
    with comparison matrices on DVE:
      rank[i] = #{j: v[j] > v[i]} + #{j < i: v[j] == v[i]}
    then a 0/1 selection matrix PT[i,k] = (rank[i] == k) applied via PE
    matmuls (with PE-transposed node-major h chunks).
  - Conv1d/maxpool/conv2/dense head all on device as small matmuls; final
    2-class softmax via sigmoid of logit differences.

Self-contained: hardcodes all shapes; no reads of /root/problem files.
"""

import sys

if "/opt/trn_rl_repo" not in sys.path:
    sys.path.insert(0, "/opt/trn_rl_repo")

import numpy as np

import concourse.bacc as bacc
import concourse.mybir as mybir
import concourse.tile as tile
from concourse.bass_utils import run_bass_kernel_spmd

F32 = mybir.dt.float32
F32R = mybir.dt.float32r

NUM_GRAPHS = 64
NPG = 512  # nodes per graph
N_TOTAL = NUM_GRAPHS * NPG
EMB = 128
DIMF = 128
NLAYERS = 4
K = 64
NCORES = 8
GPC = NUM_GRAPHS // NCORES  # graphs per core = 8
NLOC = GPC * NPG  # local nodes = 4096
LATENT = NLAYERS * DIMF + 1  # 513
DD = (K - 2) // 2 + 1  # 32
CONV2_LEN = DD - 5 + 1  # 28

# Matmul dtype for the GCN stages: F32 (safe) or F32R (fast, 11-bit operand
# mantissa). Switched by _build(); default chosen in kernel().
_NC_CACHE = {}
SECTION_MARKS = []  # (label, id_at_boundary) for profiling analysis


def _mark(nc, label):
    SECTION_MARKS.append((label, nc.next_id()))


def _round_fp32r(x):
    """Round fp32 array to the fp32r grid (RNE to 11 mantissa bits)."""
    u = np.ascontiguousarray(x, dtype=np.float32).view(np.uint32)
    bias = np.uint32(0x7FF) + ((u >> np.uint32(12)) & np.uint32(1))
    r = ((u + bias) >> np.uint32(12)) << np.uint32(12)
    return r.view(np.float32)


def _build(dt_mm, with_bias, debug):
    """Trace + compile the per-core Bass program (same on all 8 cores)."""
    nc = bacc.Bacc("TRN2", target_bir_lowering=False, debug=False,
                   num_devices=NCORES)
    DT = dt_mm

    # ---- per-core DRAM I/O ----
    H0T = nc.dram_tensor("H0T", [128, NLOC], DT, kind="ExternalInput")
    STD = nc.dram_tensor("STD", [GPC, 4, 128, NPG], DT, kind="ExternalInput")
    WC = nc.dram_tensor("WC", [NLAYERS, 128, 128], DT, kind="ExternalInput")
    W5 = nc.dram_tensor("W5", [128, 1], DT, kind="ExternalInput")
    IDN = nc.dram_tensor("IDN", [128, 128], DT, kind="ExternalInput")
    W1A = nc.dram_tensor("W1A", [4, 128, 16], F32, kind="ExternalInput")
    W1B = nc.dram_tensor("W1B", [1, 16], F32, kind="ExternalInput")
    W2T = nc.dram_tensor("W2T", [5, 16, 32], F32, kind="ExternalInput")
    D1R = nc.dram_tensor("D1R", [32, CONV2_LEN * 32], F32, kind="ExternalInput")
    D2 = nc.dram_tensor("D2", [32, 2], F32, kind="ExternalInput")
    DIFF = nc.dram_tensor("DIFF", [2, 2], F32, kind="ExternalInput")
    B1 = nc.dram_tensor("B1", [16, 1], F32, kind="ExternalInput")
    B2 = nc.dram_tensor("B2", [32, 1], F32, kind="ExternalInput")
    BD1 = nc.dram_tensor("BD1", [32, 1], F32, kind="ExternalInput")
    BD2 = nc.dram_tensor("BD2", [2, 1], F32, kind="ExternalInput")
    KI = nc.dram_tensor("KI", [128, K], F32, kind="ExternalInput")
    MJ = nc.dram_tensor("MJ", [4, 128, NPG], F32, kind="ExternalInput")
    if with_bias:
        ONE = nc.dram_tensor("ONE", [1, 128], DT, kind="ExternalInput")
        BCV = nc.dram_tensor("BCV", [NLAYERS, 1, 128], DT, kind="ExternalInput")
        B5V = nc.dram_tensor("B5V", [1, 1], DT, kind="ExternalInput")
    OUT = nc.dram_tensor("OUT", [2, GPC], F32, kind="ExternalOutput")
    if debug:
        DBG_H = nc.dram_tensor("DBG_H", [NLAYERS, 128, NLOC], F32,
                               kind="ExternalOutput")
        DBG_H5 = nc.dram_tensor("DBG_H5", [GPC, 1, NPG], F32,
                                kind="ExternalOutput")
        DBG_RANK = nc.dram_tensor("DBG_RANK", [GPC, 128, 4], F32,
                                  kind="ExternalOutput")
        DBG_POOL = nc.dram_tensor("DBG_POOL", [GPC, 128, 4 * K], F32,
                                  kind="ExternalOutput")
        DBG_POOL5 = nc.dram_tensor("DBG_POOL5", [GPC, 1, K], F32,
                                   kind="ExternalOutput")
        DBG_Y2 = nc.dram_tensor("DBG_Y2", [32, GPC * CONV2_LEN], F32,
                                kind="ExternalOutput")

    TANH = mybir.ActivationFunctionType.Tanh
    RELU = mybir.ActivationFunctionType.Relu
    SIGM = mybir.ActivationFunctionType.Sigmoid
    ADD = mybir.AluOpType.add
    MULT = mybir.AluOpType.mult
    MAX = mybir.AluOpType.max
    IS_GT = mybir.AluOpType.is_gt
    IS_EQ = mybir.AluOpType.is_equal

    with tile.TileContext(nc) as tc:
        with (
            tc.tile_pool(name="const", bufs=1) as cp,
            tc.tile_pool(name="hs", bufs=1) as hp,
            tc.tile_pool(name="st", bufs=1) as stp,
            tc.tile_pool(name="lin", bufs=8) as linp,
            tc.tile_pool(name="sc", bufs=2) as scp,
            tc.tile_pool(name="vbp", bufs=4) as vbp,
            tc.tile_pool(name="ptp", bufs=8) as ptp,
            tc.tile_pool(name="sm", bufs=4) as smp,
            tc.tile_pool(name="xs", bufs=8) as xsp,
            tc.tile_pool(name="ps512", bufs=2, space="PSUM") as ps5,
            tc.tile_pool(name="ps128", bufs=6, space="PSUM") as ps1,
        ):
            # ---- constant / weight loads (first-needed first) ----
            wc_sb = cp.tile([128, NLAYERS * 128], DT, tag="wc")
            h0 = hp.tile([128, NLOC], DT, tag="h0")
            st_sb = []
            for g in range(GPC):
                st_t = stp.tile([128, 4 * NPG], DT, tag=f"st{g}",
                                name=f"st_t{g}")
                st_sb.append(st_t)

            def load_st(g):
                for c in range(4):
                    nc.sync.dma_start(st_sb[g][:, c * NPG:(c + 1) * NPG],
                                      STD[g, c, :, :])

            # minimal first-matmul working set first: wc layer-0 slice, the
            # first h0 sliver, then graph-0's S chunks; remaining weights
            # interleave behind
            nc.sync.dma_start(wc_sb[:, 0:128], WC[0, :, :])
            for s in range(4):
                nc.sync.dma_start(h0[:, s * 128:(s + 1) * 128],
                                  H0T[:, s * 128:(s + 1) * 128])
            load_st(0)
            for l in range(1, NLAYERS):
                nc.sync.dma_start(wc_sb[:, l * 128:(l + 1) * 128], WC[l, :, :])
            for c in range(1, 4):
                nc.sync.dma_start(h0[:, c * 512:(c + 1) * 512],
                                  H0T[:, c * 512:(c + 1) * 512])
                load_st(c)
            for c in range(4, 8):
                nc.sync.dma_start(h0[:, c * 512:(c + 1) * 512],
                                  H0T[:, c * 512:(c + 1) * 512])
            for g in range(4, GPC):
                load_st(g)
            w5_sb = cp.tile([128, 1], DT, tag="w5")
            nc.sync.dma_start(w5_sb[:], W5[:])
            id_sb = cp.tile([128, 128], DT, tag="idn")
            nc.sync.dma_start(id_sb[:], IDN[:])
            if DT == F32:
                id32_sb = id_sb
            else:
                id32_sb = cp.tile([128, 128], F32, tag="idn32")
                nc.sync.dma_start(id32_sb[:], IDN[:].bitcast(F32))
            w1a_sb = cp.tile([128, 64], F32, tag="w1a")
            for c in range(4):
                nc.sync.dma_start(w1a_sb[:, c * 16:(c + 1) * 16], W1A[c, :, :])
            w1b_sb = cp.tile([1, 16], F32, tag="w1b")
            nc.sync.dma_start(w1b_sb[:], W1B[:])
            w2_sb = cp.tile([16, 160], F32, tag="w2t")
            for t5 in range(5):
                nc.sync.dma_start(w2_sb[:, t5 * 32:(t5 + 1) * 32],
                                  W2T[t5, :, :])
            d1_sb = cp.tile([32, CONV2_LEN * 32], F32, tag="d1r")
            nc.sync.dma_start(d1_sb[:], D1R[:])
            d2_sb = cp.tile([32, 2], F32, tag="d2")
            nc.sync.dma_start(d2_sb[:], D2[:])
            diff_sb = cp.tile([2, 2], F32, tag="diff")
            nc.sync.dma_start(diff_sb[:], DIFF[:])
            b1_sb = cp.tile([16, 1], F32, tag="b1")
            nc.sync.dma_start(b1_sb[:], B1[:])
            b2_sb = cp.tile([32, 1], F32, tag="b2")
            nc.sync.dma_start(b2_sb[:], B2[:])
            bd1_sb = cp.tile([32, 1], F32, tag="bd1")
            nc.sync.dma_start(bd1_sb[:], BD1[:])
            bd2_sb = cp.tile([2, 1], F32, tag="bd2")
            nc.sync.dma_start(bd2_sb[:], BD2[:])
            ki_sb = cp.tile([128, K], F32, tag="ki")
            nc.sync.dma_start(ki_sb[:], KI[:])
            mj_sb = cp.tile([128, 4 * NPG], F32, tag="mj")
            for c in range(4):
                nc.sync.dma_start(mj_sb[:, c * NPG:(c + 1) * NPG], MJ[c, :, :])
            if with_bias:
                one_sb = cp.tile([1, 128], DT, tag="one")
                nc.sync.dma_start(one_sb[:], ONE[:])
                bcv_sb = []
                for l in range(NLAYERS):
                    t = cp.tile([1, 128], DT, tag=f"bcv{l}")
                    nc.sync.dma_start(t[:], BCV[l, :, :])
                    bcv_sb.append(t)
                b5v_sb = cp.tile([1, 1], DT, tag="b5v")
                nc.sync.dma_start(b5v_sb[:], B5V[:])
            y2all = cp.tile([32, GPC * CONV2_LEN], F32, tag="y2all")

            # ---- GCN layers 1..4 ----
            _mark(nc, "layers")
            h_prev = h0
            h_layers = []
            for l in range(NLAYERS):
                _mark(nc, f"layer{l}")
                h_next = hp.tile([128, NLOC], DT, tag=f"h{l + 1}")
                for g in range(GPC):
                    lins = []
                    for cc in range(4):
                        ch = 4 * g + cc
                        lp = ps1.tile([128, 128], F32, tag="ps128")
                        nc.tensor.matmul(
                            lp[:], h_prev[:, ch * 128:(ch + 1) * 128],
                            wc_sb[:, l * 128:(l + 1) * 128],
                            start=True, stop=not with_bias)
                        if with_bias:
                            nc.tensor.matmul(lp[:], one_sb[:], bcv_sb[l][:],
                                             start=False, stop=True)
                        ln = linp.tile([128, 128], DT, tag="lin")
                        nc.vector.tensor_copy(ln[:], lp[:])
                        lins.append(ln)
                    sp = ps5.tile([128, NPG], F32, tag="ps512")
                    for cc in range(4):
                        nc.tensor.matmul(
                            sp[:], lins[cc][:],
                            st_sb[g][:, cc * NPG:(cc + 1) * NPG],
                            start=(cc == 0), stop=(cc == 3))
                    nc.scalar.activation(
                        h_next[:, g * NPG:(g + 1) * NPG], sp[:], TANH)
                h_layers.append(h_next)
                h_prev = h_next
            if debug:
                for l in range(NLAYERS):
                    nc.sync.dma_start(DBG_H[l, :, :],
                                      h_layers[l][:].bitcast(F32))

            # ---- tail, software-pipelined across graphs ----
            vcols, vbs, ptts = [], [], []

            def tail_t1(g):
                _mark(nc, f"t1_g{g}")
                # layer-5 matvec + h5 forms (vcol node-major, vb broadcast)
                l5p = ps1.tile([128, 4], F32, tag="ps128")
                for cc in range(4):
                    ch = 4 * g + cc
                    nc.tensor.matmul(
                        l5p[:, cc:cc + 1],
                        h_prev[:, ch * 128:(ch + 1) * 128], w5_sb[:],
                        start=True, stop=not with_bias)
                    if with_bias:
                        nc.tensor.matmul(l5p[:, cc:cc + 1], one_sb[:],
                                         b5v_sb[:], start=False, stop=True)
                lin5 = smp.tile([128, 4], DT, tag="lin5")
                nc.vector.tensor_copy(lin5[:], l5p[:])
                # msg5 = S @ lin5 computed d-major: 16 tiny N=1 matmuls
                m5p = ps1.tile([128, 4], F32, tag="ps128")
                for dc in range(4):
                    for sc in range(4):
                        nc.tensor.matmul(
                            m5p[:, dc:dc + 1],
                            st_sb[g][:, sc * NPG + dc * 128:
                                      sc * NPG + (dc + 1) * 128],
                            lin5[:, sc:sc + 1],
                            start=(sc == 0), stop=(sc == 3))
                # vcol = h5 node-major directly
                vcol = ptp.tile([128, 4], F32, tag="vcol")
                nc.scalar.activation(vcol[:], m5p[:], TANH)
                # h5 row form: PE-transpose vcol, copy rows to SBUF
                vtp = ps1.tile([4, 128], F32, tag="ps128")
                nc.tensor.transpose(vtp[:], vcol[:], id32_sb[:])
                vts = smp.tile([4, 128], F32, tag="vts")
                nc.vector.tensor_copy(vts[:], vtp[:])
                h5r = smp.tile([1, NPG], F32, tag="h5r")
                for cc in range(4):
                    nc.sync.dma_start(h5r[0:1, cc * 128:(cc + 1) * 128],
                                      vts[cc:cc + 1, :])
                # v broadcast (exact copy to all partitions)
                vb = vbp.tile([128, NPG], F32, tag="vb")
                nc.gpsimd.partition_broadcast(vb[:], h5r[0:1, :])
                vcols.append(vcol)
                vbs.append(vb)
                if debug:
                    nc.sync.dma_start(DBG_H5[g, :, :], h5r[:])

            def tail_t2(g):
                _mark(nc, f"t2_g{g}")
                # exact stable ranks + selection matrix
                vb, vcol = vbs[g], vcols[g]
                rank = smp.tile([128, 4], F32, tag="rank")
                for cc in range(4):
                    t1 = scp.tile([128, NPG], F32, tag="tt")
                    ra = smp.tile([128, 2], F32, tag="ra")
                    nc.vector.tensor_scalar(
                        out=t1[:], in0=vb[:], scalar1=vcol[:, cc:cc + 1],
                        scalar2=None, op0=IS_GT, op1=ADD,
                        accum_out=ra[:, 0:1])
                    t2 = scp.tile([128, NPG], F32, tag="tt")
                    nc.vector.scalar_tensor_tensor(
                        out=t2[:], in0=vb[:], scalar=vcol[:, cc:cc + 1],
                        in1=mj_sb[:, cc * NPG:(cc + 1) * NPG],
                        op0=IS_EQ, op1=MULT, accum_out=ra[:, 1:2])
                    nc.vector.tensor_tensor(
                        out=rank[:, cc:cc + 1], in0=ra[:, 0:1],
                        in1=ra[:, 1:2], op=ADD)
                if debug:
                    nc.sync.dma_start(DBG_RANK[g, :, :], rank[:])
                ptt = ptp.tile([128, 4 * K], F32, tag="pt")
                for cc in range(4):
                    nc.vector.tensor_scalar(
                        out=ptt[:, cc * K:(cc + 1) * K], in0=ki_sb[:],
                        scalar1=rank[:, cc:cc + 1], scalar2=None, op0=IS_EQ)
                ptts.append(ptt)

            # T3: pooling (PE transpose + selection matmuls) and conv head
            pooleds = {}

            def tail_t3(g):
                _mark(nc, f"t3_g{g}")
                ptt, vcol = ptts[g], vcols[g]
                pooledT = scp.tile([128, 4 * K], F32, tag="pooled", bufs=3)
                for l in range(NLAYERS):
                    xts = []
                    for cc in range(4):
                        ch = 4 * g + cc
                        xp = ps1.tile([128, 128], DT, tag="ps128")
                        nc.tensor.transpose(
                            xp[:], h_layers[l][:, ch * 128:(ch + 1) * 128],
                            id_sb[:])
                        xt = xsp.tile([128, 128], F32, tag="x")
                        nc.any.tensor_copy(xt[:], xp[:])
                        xts.append(xt)
                    pp = ps1.tile([128, K], F32, tag="ps128")
                    for cc in range(4):
                        nc.tensor.matmul(pp[:], xts[cc][:],
                                         ptt[:, cc * K:(cc + 1) * K],
                                         start=(cc == 0), stop=(cc == 3))
                    nc.any.tensor_copy(pooledT[:, l * K:(l + 1) * K], pp[:])
                p5p = ps1.tile([1, K], F32, tag="ps128")
                for cc in range(4):
                    nc.tensor.matmul(p5p[:], vcol[:, cc:cc + 1],
                                     ptt[:, cc * K:(cc + 1) * K],
                                     start=(cc == 0), stop=(cc == 3))
                pool5 = smp.tile([1, K], F32, tag="pool5")
                nc.any.tensor_copy(pool5[:], p5p[:])
                if debug:
                    nc.sync.dma_start(DBG_POOL[g, :, :], pooledT[:])
                    nc.sync.dma_start(DBG_POOL5[g, :, :], pool5[:])
                pooleds[g] = (pooledT, pool5)

            def tail_t3b(g):
                _mark(nc, f"t3b_g{g}")
                pooledT, pool5 = pooleds[g]
                # head: conv1 (1x513 stride-513) -> relu -> maxpool2
                y1p = ps1.tile([16, K], F32, tag="ps128")
                for cc in range(4):
                    nc.tensor.matmul(y1p[:], w1a_sb[:, cc * 16:(cc + 1) * 16],
                                     pooledT[:, cc * K:(cc + 1) * K],
                                     start=(cc == 0), stop=False)
                nc.tensor.matmul(y1p[:], w1b_sb[:], pool5[:],
                                 start=False, stop=True)
                y1 = smp.tile([16, K], F32, tag="y1")
                nc.scalar.activation(y1[:], y1p[:], RELU, bias=b1_sb[:, 0:1])
                mp = smp.tile([16, K // 2], F32, tag="mp")
                y1v = y1[:].rearrange("p (a b) -> p a b", b=2)
                nc.vector.tensor_tensor(out=mp[:], in0=y1v[:, :, 0:1],
                                        in1=y1v[:, :, 1:2], op=MAX)

                # conv2 (kernel 5) -> relu
                y2p = ps1.tile([32, CONV2_LEN], F32, tag="ps128")
                for t5 in range(5):
                    nc.tensor.matmul(y2p[:], w2_sb[:, t5 * 32:(t5 + 1) * 32],
                                     mp[:, t5:t5 + CONV2_LEN],
                                     start=(t5 == 0), stop=(t5 == 4))
                nc.scalar.activation(
                    y2all[:, g * CONV2_LEN:(g + 1) * CONV2_LEN], y2p[:],
                    RELU, bias=b2_sb[:, 0:1])

            # emit interleaved: T1(g), T2(g-2), T3 pooling+head (g-4)
            for g in range(GPC):
                tail_t1(g)
                if g >= 2:
                    tail_t2(g - 2)
                if g >= 4:
                    tail_t3(g - 4)
                    tail_t3b(g - 4)
            for g in range(GPC - 2, GPC):
                tail_t2(g)
            for g in range(GPC - 4, GPC):
                tail_t3(g)
                tail_t3b(g)

            if debug:
                nc.sync.dma_start(DBG_Y2[:], y2all[:])

            _mark(nc, "densetail")
            # ---- core-level dense tail (batched over the 8 graphs) ----
            h1p = ps1.tile([32, GPC], F32, tag="ps128")
            y2v = y2all[:].rearrange("p (g t) -> p g t", t=CONV2_LEN)
            for t5 in range(CONV2_LEN):
                nc.tensor.matmul(h1p[:], d1_sb[:, t5 * 32:(t5 + 1) * 32],
                                 y2v[:, :, t5:t5 + 1],
                                 start=(t5 == 0), stop=(t5 == CONV2_LEN - 1))
            h1s = smp.tile([32, GPC], F32, tag="h1s")
            nc.scalar.activation(h1s[:], h1p[:], RELU, bias=bd1_sb[:, 0:1])
            lgp = ps1.tile([2, GPC], F32, tag="ps128")
            nc.tensor.matmul(lgp[:], d2_sb[:], h1s[:], start=True, stop=True)
            lg = smp.tile([2, GPC], F32, tag="lg")
            nc.vector.tensor_scalar(out=lg[:], in0=lgp[:],
                                    scalar1=bd2_sb[:, 0:1], scalar2=None,
                                    op0=ADD)
            dfp = ps1.tile([2, GPC], F32, tag="ps128")
            nc.tensor.matmul(dfp[:], diff_sb[:], lg[:], start=True, stop=True)
            pr = smp.tile([2, GPC], F32, tag="pr")
            nc.scalar.activation(pr[:], dfp[:], SIGM)
            nc.sync.dma_start(OUT[:], pr[:])

    nc.compile()
    return nc


def _get_nc(dt_key, with_bias, debug):
    key = (dt_key, with_bias, debug)
    if key not in _NC_CACHE:
        dt_mm = F32R if dt_key == "f32r" else F32
        _NC_CACHE[key] = _build(dt_mm, with_bias, debug)
    return _NC_CACHE[key]


def prepare_host(inputs, dt_key):
    """All host-side index preprocessing + per-core input maps."""
    x = np.asarray(inputs["x"]).astype(np.int64)
    edge_index = np.asarray(inputs["edge_index"]).astype(np.int64)
    emb = np.ascontiguousarray(np.asarray(inputs["emb"], dtype=np.float32))
    W_convs = np.asarray(inputs["W_convs"], dtype=np.float32)
    b_convs = np.asarray(inputs["b_convs"], dtype=np.float32)
    W_last = np.asarray(inputs["W_last"], dtype=np.float32)
    b_last = np.asarray(inputs["b_last"], dtype=np.float32)
    conv1_w = np.asarray(inputs["conv1_w"], dtype=np.float32)
    conv1_b = np.asarray(inputs["conv1_b"], dtype=np.float32)
    conv2_w = np.asarray(inputs["conv2_w"], dtype=np.float32)
    conv2_b = np.asarray(inputs["conv2_b"], dtype=np.float32)
    d1_w = np.asarray(inputs["d1_w"], dtype=np.float32)
    d1_b = np.asarray(inputs["d1_b"], dtype=np.float32)
    d2_w = np.asarray(inputs["d2_w"], dtype=np.float32)
    d2_b = np.asarray(inputs["d2_b"], dtype=np.float32)

    src, dst = edge_index[0], edge_index[1]
    deg = (np.bincount(src, minlength=N_TOTAL) + 1).astype(np.float32)
    invdeg = (np.float32(1.0) / deg).astype(np.float32)
    gid = dst >> 9
    flat = (gid * NPG + (dst & 511)) * NPG + (src & 511)
    A = np.bincount(flat, minlength=NUM_GRAPHS * NPG * NPG)
    A = A.astype(np.float32).reshape(NUM_GRAPHS, NPG, NPG)
    idx = np.arange(NPG)
    A[:, idx, idx] += 1.0
    S = A * invdeg.reshape(NUM_GRAPHS, NPG, 1)
    ST = np.ascontiguousarray(S.transpose(0, 2, 1)).reshape(
        NUM_GRAPHS, 4, 128, NPG)

    h0 = emb[x]  # [N, 128]

    rnd = _round_fp32r if dt_key == "f32r" else (lambda a: a)
    with_bias = bool(np.any(b_convs) or np.any(b_last))

    w1 = np.ascontiguousarray(conv1_w[:, 0, :].T)  # [513, 16]
    shared = {
        "WC": rnd(np.ascontiguousarray(W_convs)),
        "W5": rnd(np.ascontiguousarray(W_last)),
        "IDN": rnd(np.eye(128, dtype=np.float32)),
        "W1A": np.ascontiguousarray(w1[:512].reshape(4, 128, 16)),
        "W1B": np.ascontiguousarray(w1[512:513]),
        "W2T": np.ascontiguousarray(conv2_w.transpose(2, 1, 0)),
        "D1R": np.ascontiguousarray(d1_w.reshape(DD, CONV2_LEN * 32)
                                    .astype(np.float32)),
        "D2": np.ascontiguousarray(d2_w),
        "DIFF": np.array([[1.0, -1.0], [-1.0, 1.0]], dtype=np.float32),
        "B1": np.ascontiguousarray(conv1_b.reshape(16, 1)),
        "B2": np.ascontiguousarray(conv2_b.reshape(32, 1)),
        "BD1": np.ascontiguousarray(d1_b.reshape(32, 1)),
        "BD2": np.ascontiguousarray(d2_b.reshape(2, 1)),
        "KI": np.ascontiguousarray(
            np.broadcast_to(np.arange(K, dtype=np.float32), (128, K))),
        "MJ": np.ascontiguousarray(
            (np.arange(NPG)[None, None, :]
             < (np.arange(4)[:, None, None] * 128
                + np.arange(128)[None, :, None])).astype(np.float32)),
    }
    if with_bias:
        shared["ONE"] = rnd(np.ones((1, 128), dtype=np.float32))
        shared["BCV"] = rnd(np.ascontiguousarray(
            b_convs.reshape(NLAYERS, 1, 128)))
        shared["B5V"] = rnd(np.ascontiguousarray(b_last.reshape(1, 1)))

    in_maps = []
    for c in range(NCORES):
        h0c = np.ascontiguousarray(h0[c * NLOC:(c + 1) * NLOC].T)
        m = dict(shared)
        m["H0T"] = rnd(h0c)
        m["STD"] = rnd(np.ascontiguousarray(ST[c * GPC:(c + 1) * GPC]))
        in_maps.append(m)
    return in_maps, with_bias


def run(inputs, dt_key="f32", debug=False, **spmd_kwargs):
    in_maps, with_bias = prepare_host(inputs, dt_key)
    nc = _get_nc(dt_key, with_bias, debug)
    res = run_bass_kernel_spmd(nc, in_maps, core_ids=list(range(NCORES)),
                               **spmd_kwargs)
    out = np.empty((NUM_GRAPHS, 2), dtype=np.float32)
    for c in range(NCORES):
        out[c * GPC:(c + 1) * GPC, :] = res.results[c]["OUT"].T
    return out, res


def kernel(**inputs):
    out, _ = run(inputs, dt_key="f32")
    return out



# revision 15
# speedup vs baseline: 1.3286x; 1.3286x over previous
"""DGCNN (gnn_message_passing) Trainium2 Bass kernel, v2.

Strategy (data-parallel over graphs, 8 graphs per NeuronCore):
  - Host builds, per graph, the INTEGER operator A^T where A = adj-multiplicity
    + I (entries are small ints, exactly representable in bf16), shipped as
    4 chunks of [128, 512] bf16. The degree normalization inv = 1/deg is
    applied on-device AFTER the matmul (elementwise, exact fp32), so the
    aggregation matmul never rounds the operator.
  - Each GCN layer, per graph:
      lin  = h @ W          (fp32 matmuls, exact, node-major chunks)
      hi   = bf16(lin); lo = bf16(lin - hi)   (ACT copy + DVE subtract)
      u    = A^T-chunks x (hi | lo) -> PSUM [128f, 512d]  (8 bf16 matmuls at
             1 cyc/row -- 2x faster than one fp32 matmul, fp32-exact result)
      msg  = u * invb       (Pool elementwise)
      h'   = tanh(msg)      (ACT)
    This reproduces the reference to ~3e-7 (verified in numpy: zero top-k
    rank flips).
  - Layer 5 (h5): fp32/bf16-comp matvecs against the same A^T chunks,
    inv scale + tanh node-major, then transpose -> row form -> broadcast.
  - Ranks: exact stable rank[i] = #{v>v_i} + #{j<i: v==v_i} via DVE(is_gt)
    + Pool(is_eq*mask) passes; one-hot selection matrix PT from ranks.
  - Head (conv1-first): y1[o,d] = relu(W1 . xcat[:,d]) computed for ALL 512
    columns per graph via 4 f32r matmuls (free=512) + DVE rank-1 h5 update;
    then the CHEAP [16,512] tensor is transposed (4 tiny PE transposes) and
    64 columns selected by one-hot matmuls -- this replaces transposing all
    4 [128,512] h matrices per graph of the old design.
  - maxpool/conv2/dense head as small fp32 matmuls; final 2-class softmax
    via sigmoid of logit differences.

Modes: dt_key "f32" = compensated (default, ~3e-7 rel err);
       "f32r" = 1-term f32r aggregation (faster, ~1e-2 rel err).

Self-contained: hardcodes all shapes; no reads of /root/problem files.
"""

import sys

if "/opt/trn_rl_repo" not in sys.path:
    sys.path.insert(0, "/opt/trn_rl_repo")

import ml_dtypes
import numpy as np

import concourse.bacc as bacc
import concourse.mybir as mybir
import concourse.tile as tile
from concourse.bass_utils import run_bass_kernel_spmd

F32 = mybir.dt.float32
F32R = mybir.dt.float32r
BF16 = mybir.dt.bfloat16

NUM_GRAPHS = 64
NPG = 512  # nodes per graph
N_TOTAL = NUM_GRAPHS * NPG
EMB = 128
DIMF = 128
NLAYERS = 4
K = 64
NCORES = 8
GPC = NUM_GRAPHS // NCORES  # graphs per core = 8
NLOC = GPC * NPG  # local nodes = 4096
LATENT = NLAYERS * DIMF + 1  # 513
DD = (K - 2) // 2 + 1  # 32
CONV2_LEN = DD - 5 + 1  # 28

_NC_CACHE = {}
SECTION_MARKS = []  # (label, id_at_boundary) for profiling analysis


def _mark(nc, label):
    SECTION_MARKS.append((label, nc.next_id()))


def _build(mode, with_bias, debug):
    """Trace + compile the per-core Bass program (same on all 8 cores).

    mode: "comp" (bf16 A + hi/lo compensated agg, near-exact) or
          "fast" (f32r A + single rounded agg).
    """
    comp = mode == "comp"
    DT_A = BF16 if comp else F32R

    nc = bacc.Bacc("TRN2", target_bir_lowering=False, debug=False,
                   num_devices=NCORES)

    # ---- per-core DRAM I/O ----
    H0T = nc.dram_tensor("H0T", [128, NLOC], F32, kind="ExternalInput")
    ATD = nc.dram_tensor("ATD", [GPC, 4, 128, NPG], DT_A,
                         kind="ExternalInput")
    WC = nc.dram_tensor("WC", [NLAYERS, 128, 128], F32, kind="ExternalInput")
    W5 = nc.dram_tensor("W5", [128, 1], F32, kind="ExternalInput")
    INVR = nc.dram_tensor("INVR", [GPC, 1, NPG], F32, kind="ExternalInput")
    INV4 = nc.dram_tensor("INV4", [GPC, 128, 4], F32, kind="ExternalInput")
    IDN = nc.dram_tensor("IDN", [128, 128], F32, kind="ExternalInput")
    KI = nc.dram_tensor("KI", [128, K], F32, kind="ExternalInput")
    MJ = nc.dram_tensor("MJ", [4, 128, NPG], F32, kind="ExternalInput")
    W1A = nc.dram_tensor("W1A", [4, 128, 16], F32, kind="ExternalInput")
    W1BR = nc.dram_tensor("W1BR", [1, 16], F32, kind="ExternalInput")
    B1R = nc.dram_tensor("B1R", [1, 16], F32, kind="ExternalInput")
    ONER = nc.dram_tensor("ONER", [1, 128], F32, kind="ExternalInput")
    W2T = nc.dram_tensor("W2T", [5, 16, 32], F32, kind="ExternalInput")
    B2 = nc.dram_tensor("B2", [32, 1], F32, kind="ExternalInput")
    D1R = nc.dram_tensor("D1R", [32, CONV2_LEN * 32], F32,
                         kind="ExternalInput")
    BD1 = nc.dram_tensor("BD1", [32, 1], F32, kind="ExternalInput")
    D2 = nc.dram_tensor("D2", [32, 2], F32, kind="ExternalInput")
    BD2 = nc.dram_tensor("BD2", [2, 1], F32, kind="ExternalInput")
    DIFF = nc.dram_tensor("DIFF", [2, 2], F32, kind="ExternalInput")
    if with_bias:
        ONE = nc.dram_tensor("ONE", [1, 128], F32, kind="ExternalInput")
        BCV = nc.dram_tensor("BCV", [NLAYERS, 1, 128], F32,
                             kind="ExternalInput")
        B5V = nc.dram_tensor("B5V", [1, 1], F32, kind="ExternalInput")
    OUT = nc.dram_tensor("OUT", [2, GPC], F32, kind="ExternalOutput")
    if debug:
        DBG_H = nc.dram_tensor("DBG_H", [NLAYERS, 128, NLOC], F32,
                               kind="ExternalOutput")
        DBG_H5 = nc.dram_tensor("DBG_H5", [GPC, 1, NPG], F32,
                                kind="ExternalOutput")
        DBG_RANK = nc.dram_tensor("DBG_RANK", [GPC, 128, 4], F32,
                                  kind="ExternalOutput")
        DBG_Y1 = nc.dram_tensor("DBG_Y1", [GPC, 128, 4 * 16], F32,
                                kind="ExternalOutput")
        DBG_SEL = nc.dram_tensor("DBG_SEL", [GPC, 16, K], F32,
                                 kind="ExternalOutput")
        DBG_Y2 = nc.dram_tensor("DBG_Y2", [32, GPC * CONV2_LEN], F32,
                                kind="ExternalOutput")

    TANH = mybir.ActivationFunctionType.Tanh
    RELU = mybir.ActivationFunctionType.Relu
    SIGM = mybir.ActivationFunctionType.Sigmoid
    ADD = mybir.AluOpType.add
    SUB = mybir.AluOpType.subtract
    MULT = mybir.AluOpType.mult
    MAX = mybir.AluOpType.max
    IS_GT = mybir.AluOpType.is_gt
    IS_EQ = mybir.AluOpType.is_equal

    with tile.TileContext(nc) as tc:
        with (
            tc.tile_pool(name="const", bufs=1) as cp,
            tc.tile_pool(name="hs", bufs=4) as hp,
            tc.tile_pool(name="at", bufs=1) as atp,
            tc.tile_pool(name="iv", bufs=1) as ivp,
            tc.tile_pool(name="hi", bufs=8) as hip,
            tc.tile_pool(name="lo", bufs=8) as lop,
            tc.tile_pool(name="sc5", bufs=3) as sc5p,
            tc.tile_pool(name="vbp", bufs=4) as vbp,
            tc.tile_pool(name="ptp", bufs=4) as ptp,
            tc.tile_pool(name="sm", bufs=6) as smp,
            tc.tile_pool(name="scr", bufs=3) as scp,
            tc.tile_pool(name="y1", bufs=4) as y1p_pool,
            tc.tile_pool(name="ps512", bufs=2, space="PSUM") as ps5,
            tc.tile_pool(name="ps128", bufs=6, space="PSUM") as ps1,
        ):
            # ---- constant / weight loads (first-needed first) ----
            wc_sb = cp.tile([128, NLAYERS * 128], F32, tag="wc")
            h0 = hp.tile([128, NLOC], F32, tag="h")
            at_sb = []
            for g in range(GPC):
                at_t = atp.tile([128, 4 * NPG], DT_A, tag=f"at{g}",
                                name=f"at_t{g}")
                at_sb.append(at_t)

            def load_at(g):
                for c in range(4):
                    nc.sync.dma_start(at_sb[g][:, c * NPG:(c + 1) * NPG],
                                      ATD[g, c, :, :])

            invb = []
            for g in range(GPC):
                t = ivp.tile([128, NPG], F32, tag=f"invb{g}",
                             name=f"invb{g}")
                invb.append(t)

            def load_inv(g):
                ir = smp.tile([1, NPG], F32, tag="inr")
                nc.sync.dma_start(ir[0:1, :], INVR[g, :, :])
                nc.gpsimd.partition_broadcast(invb[g][:], ir[0:1, :])

            # minimal first-matmul working set first: wc layer-0 slice, the
            # first h0 sliver, then graph-0's A chunks + inv; remaining
            # weights interleave behind
            nc.sync.dma_start(wc_sb[:, 0:128], WC[0, :, :])
            for s in range(4):
                nc.sync.dma_start(h0[:, s * 128:(s + 1) * 128],
                                  H0T[:, s * 128:(s + 1) * 128])
            load_at(0)
            load_inv(0)
            for l in range(1, NLAYERS):
                nc.sync.dma_start(wc_sb[:, l * 128:(l + 1) * 128], WC[l, :, :])
            for c in range(1, 4):
                nc.sync.dma_start(h0[:, c * 512:(c + 1) * 512],
                                  H0T[:, c * 512:(c + 1) * 512])
                load_at(c)
                load_inv(c)
            for c in range(4, 8):
                nc.sync.dma_start(h0[:, c * 512:(c + 1) * 512],
                                  H0T[:, c * 512:(c + 1) * 512])
            for g in range(4, GPC):
                load_at(g)
                load_inv(g)
            w5_sb = cp.tile([128, 1], F32, tag="w5")
            nc.sync.dma_start(w5_sb[:], W5[:])
            inv4_sb = cp.tile([128, 4 * GPC], F32, tag="inv4")
            for g in range(GPC):
                nc.sync.dma_start(inv4_sb[:, g * 4:(g + 1) * 4],
                                  INV4[g, :, :])
            id_sb = cp.tile([128, 128], F32, tag="idn")
            nc.sync.dma_start(id_sb[:], IDN[:])
            ki_sb = cp.tile([128, K], F32, tag="ki")
            nc.sync.dma_start(ki_sb[:], KI[:])
            mj_sb = cp.tile([128, 4 * NPG], F32, tag="mj")
            for c in range(4):
                nc.sync.dma_start(mj_sb[:, c * NPG:(c + 1) * NPG], MJ[c, :, :])
            w1a_sb = cp.tile([128, 64], F32, tag="w1a")
            for c in range(4):
                nc.sync.dma_start(w1a_sb[:, c * 16:(c + 1) * 16], W1A[c, :, :])
            b1r_sb = cp.tile([1, 16], F32, tag="b1r")
            nc.sync.dma_start(b1r_sb[:], B1R[:])
            oner_sb = cp.tile([1, 128], F32, tag="oner")
            nc.sync.dma_start(oner_sb[:], ONER[:])
            w1br_sb = cp.tile([1, 16], F32, tag="w1br")
            nc.sync.dma_start(w1br_sb[:], W1BR[:])
            w1bb = cp.tile([128, 16], F32, tag="w1bb")
            nc.gpsimd.partition_broadcast(w1bb[:], w1br_sb[0:1, :])
            w2_sb = cp.tile([16, 160], F32, tag="w2t")
            for t5 in range(5):
                nc.sync.dma_start(w2_sb[:, t5 * 32:(t5 + 1) * 32],
                                  W2T[t5, :, :])
            b2_sb = cp.tile([32, 1], F32, tag="b2")
            nc.sync.dma_start(b2_sb[:], B2[:])
            d1_sb = cp.tile([32, CONV2_LEN * 32], F32, tag="d1r")
            nc.sync.dma_start(d1_sb[:], D1R[:])
            bd1_sb = cp.tile([32, 1], F32, tag="bd1")
            nc.sync.dma_start(bd1_sb[:], BD1[:])
            d2_sb = cp.tile([32, 2], F32, tag="d2")
            nc.sync.dma_start(d2_sb[:], D2[:])
            bd2_sb = cp.tile([2, 1], F32, tag="bd2")
            nc.sync.dma_start(bd2_sb[:], BD2[:])
            diff_sb = cp.tile([2, 2], F32, tag="diff")
            nc.sync.dma_start(diff_sb[:], DIFF[:])
            if with_bias:
                one_sb = cp.tile([1, 128], F32, tag="one")
                nc.sync.dma_start(one_sb[:], ONE[:])
                bcv_sb = []
                for l in range(NLAYERS):
                    t = cp.tile([1, 128], F32, tag=f"bcv{l}")
                    nc.sync.dma_start(t[:], BCV[l, :, :])
                    bcv_sb.append(t)
                b5v_sb = cp.tile([1, 1], F32, tag="b5v")
                nc.sync.dma_start(b5v_sb[:], B5V[:])
            y2all = cp.tile([32, GPC * CONV2_LEN], F32, tag="y2all")

            h_layers = []

            def layer_graph(l, g, h_prev, h_next):
                """One GCN layer for one graph: lin -> split -> agg ->
                inv-scale -> tanh."""
                his, los = [], []
                for cc in range(4):
                    ch = 4 * g + cc
                    lp = ps1.tile([128, 128], F32, tag="ps128")
                    nc.tensor.matmul(
                        lp[:], h_prev[:, ch * 128:(ch + 1) * 128],
                        wc_sb[:, l * 128:(l + 1) * 128],
                        start=True, stop=not with_bias)
                    if with_bias:
                        nc.tensor.matmul(lp[:], one_sb[:], bcv_sb[l][:],
                                         start=False, stop=True)
                    if comp:
                        hi = hip.tile([128, 128], BF16, tag="hi")
                        nc.scalar.copy(hi[:], lp[:])
                        lo = lop.tile([128, 128], BF16, tag="lo")
                        nc.vector.tensor_tensor(out=lo[:], in0=lp[:],
                                                in1=hi[:], op=SUB)
                        his.append(hi)
                        los.append(lo)
                    else:
                        lr = hip.tile([128, 128], F32R, tag="hi")
                        nc.vector.tensor_copy(lr[:], lp[:])
                        his.append(lr)
                sp = ps5.tile([128, NPG], F32, tag="ps512")
                nmm = 8 if comp else 4
                i = 0
                for arr in ([his, los] if comp else [his]):
                    for cc in range(4):
                        nc.tensor.matmul(
                            sp[:], arr[cc][:],
                            at_sb[g][:, cc * NPG:(cc + 1) * NPG],
                            start=(i == 0), stop=(i == nmm - 1))
                        i += 1
                sc5 = sc5p.tile([128, NPG], F32, tag="sc5")
                nc.vector.tensor_tensor(out=sc5[:], in0=sp[:],
                                        in1=invb[g][:], op=MULT)
                nc.scalar.activation(
                    h_next[:, g * NPG:(g + 1) * NPG], sc5[:], TANH)

            # ---- tail stages, software-pipelined across graphs ----
            vcols, vbs, ptts, y1ts, sels = {}, {}, {}, {}, {}

            def tail_t1(g):
                """h5 for graph g: fp32/bf16-comp matvecs + inv + tanh,
                then row form + partition broadcast."""
                _mark(nc, f"t1_g{g}")
                h4 = h_layers[NLAYERS - 1]
                l5p = ps1.tile([128, 4], F32, tag="ps128")
                for cc in range(4):
                    ch = 4 * g + cc
                    nc.tensor.matmul(
                        l5p[:, cc:cc + 1],
                        h4[:, ch * 128:(ch + 1) * 128], w5_sb[:],
                        start=True, stop=not with_bias)
                    if with_bias:
                        nc.tensor.matmul(l5p[:, cc:cc + 1], one_sb[:],
                                         b5v_sb[:], start=False, stop=True)
                m5p = ps1.tile([128, 4], F32, tag="ps128")
                if comp:
                    hi5 = smp.tile([128, 4], BF16, tag="hi5")
                    nc.scalar.copy(hi5[:], l5p[:])
                    lo5 = smp.tile([128, 4], BF16, tag="lo5")
                    nc.vector.tensor_tensor(out=lo5[:], in0=l5p[:],
                                            in1=hi5[:], op=SUB)
                    for dc in range(4):
                        i = 0
                        for arr in (hi5, lo5):
                            for sc in range(4):
                                nc.tensor.matmul(
                                    m5p[:, dc:dc + 1],
                                    at_sb[g][:, sc * NPG + dc * 128:
                                             sc * NPG + (dc + 1) * 128],
                                    arr[:, sc:sc + 1],
                                    start=(i == 0), stop=(i == 7))
                                i += 1
                else:
                    lin5 = smp.tile([128, 4], F32, tag="lin5")
                    nc.vector.tensor_copy(lin5[:], l5p[:])
                    for dc in range(4):
                        for sc in range(4):
                            nc.tensor.matmul(
                                m5p[:, dc:dc + 1],
                                at_sb[g][:, sc * NPG + dc * 128:
                                         sc * NPG + (dc + 1) * 128]
                                .bitcast(F32),
                                lin5[:, sc:sc + 1],
                                start=(sc == 0), stop=(sc == 3))
                vtm = smp.tile([128, 4], F32, tag="vtm")
                nc.vector.tensor_tensor(out=vtm[:], in0=m5p[:],
                                        in1=inv4_sb[:, g * 4:(g + 1) * 4],
                                        op=MULT)
                vcol = smp.tile([128, 4], F32, tag="vcol")
                nc.scalar.activation(vcol[:], vtm[:], TANH)
                # row form: PE-transpose vcol, copy rows to SBUF
                vtp = ps1.tile([4, 128], F32, tag="ps128")
                nc.tensor.transpose(vtp[:], vcol[:], id_sb[:])
                vts = smp.tile([4, 128], F32, tag="vts")
                nc.vector.tensor_copy(vts[:], vtp[:])
                h5r = smp.tile([1, NPG], F32, tag="h5r")
                for cc in range(4):
                    nc.sync.dma_start(h5r[0:1, cc * 128:(cc + 1) * 128],
                                      vts[cc:cc + 1, :])
                vb = vbp.tile([128, NPG], F32, tag="vb")
                nc.gpsimd.partition_broadcast(vb[:], h5r[0:1, :])
                vcols[g] = vcol
                vbs[g] = vb
                if debug:
                    nc.sync.dma_start(DBG_H5[g, :, :], h5r[:])

            def tail_t2(g):
                """Exact stable ranks + one-hot selection matrix."""
                _mark(nc, f"t2_g{g}")
                vb, vcol = vbs[g], vcols[g]
                rank = smp.tile([128, 4], F32, tag="rank")
                for cc in range(4):
                    t1s = scp.tile([128, NPG], F32, tag="tt")
                    ra = smp.tile([128, 2], F32, tag="ra")
                    nc.vector.tensor_scalar(
                        out=t1s[:], in0=vb[:], scalar1=vcol[:, cc:cc + 1],
                        scalar2=None, op0=IS_GT, op1=ADD,
                        accum_out=ra[:, 0:1])
                    t2s = scp.tile([128, NPG], F32, tag="tt")
                    nc.vector.scalar_tensor_tensor(
                        out=t2s[:], in0=vb[:], scalar=vcol[:, cc:cc + 1],
                        in1=mj_sb[:, cc * NPG:(cc + 1) * NPG],
                        op0=IS_EQ, op1=MULT, accum_out=ra[:, 1:2])
                    nc.vector.tensor_tensor(
                        out=rank[:, cc:cc + 1], in0=ra[:, 0:1],
                        in1=ra[:, 1:2], op=ADD)
                if debug:
                    nc.sync.dma_start(DBG_RANK[g, :, :], rank[:])
                ptt = ptp.tile([128, 4 * K], F32, tag="pt")
                for cc in range(4):
                    nc.vector.tensor_scalar(
                        out=ptt[:, cc * K:(cc + 1) * K], in0=ki_sb[:],
                        scalar1=rank[:, cc:cc + 1], scalar2=None, op0=IS_EQ)
                ptts[g] = ptt

            def tail_t3(g):
                """conv1-first, node-major: y1T[d,o] = relu(xcat[:,d].W1 + b1)
                for all 512 nodes via fp32 matmuls with h chunks stationary
                (free=16), h5 rank-1 term via per-partition DVE scalar op,
                then one-hot row selection -> sel [16, K]. All exact fp32."""
                _mark(nc, f"t3_g{g}")
                vcol, ptt = vcols[g], ptts[g]
                y1t = y1p_pool.tile([128, 4 * 16], F32, tag="y1t")
                for cc in range(4):
                    ch = 4 * g + cc
                    p = ps1.tile([128, 16], F32, tag="ps128")
                    for l in range(NLAYERS):
                        nc.tensor.matmul(
                            p[:], h_layers[l][:, ch * 128:(ch + 1) * 128],
                            w1a_sb[:, l * 16:(l + 1) * 16],
                            start=(l == 0), stop=False)
                    nc.tensor.matmul(p[:], oner_sb[:], b1r_sb[:],
                                     start=False, stop=True)
                    q = y1p_pool.tile([128, 16], F32, tag="y1q")
                    nc.vector.scalar_tensor_tensor(
                        out=q[:], in0=w1bb[:], scalar=vcol[:, cc:cc + 1],
                        in1=p[:], op0=MULT, op1=ADD)
                    nc.scalar.activation(y1t[:, cc * 16:(cc + 1) * 16],
                                         q[:], RELU)
                if debug:
                    nc.sync.dma_start(DBG_Y1[g, :, :], y1t[:])
                # select 64 rows (nodes) via one-hot matmuls
                selp = ps1.tile([16, K], F32, tag="ps128")
                for cc in range(4):
                    nc.tensor.matmul(selp[:], y1t[:, cc * 16:(cc + 1) * 16],
                                     ptt[:, cc * K:(cc + 1) * K],
                                     start=(cc == 0), stop=(cc == 3))
                sel = smp.tile([16, K], F32, tag="sel")
                nc.any.tensor_copy(sel[:], selp[:])
                sels[g] = sel
                if debug:
                    nc.sync.dma_start(DBG_SEL[g, :, :], sel[:])

            def tail_t3b(g):
                """maxpool(2) -> conv2 -> relu into y2all."""
                _mark(nc, f"t3b_g{g}")
                sel = sels[g]
                mp = smp.tile([16, K // 2], F32, tag="mp")
                selv = sel[:].rearrange("p (a b) -> p a b", b=2)
                nc.vector.tensor_tensor(out=mp[:], in0=selv[:, :, 0:1],
                                        in1=selv[:, :, 1:2], op=MAX)
                y2p = ps1.tile([32, CONV2_LEN], F32, tag="ps128")
                for t5 in range(5):
                    nc.tensor.matmul(y2p[:], w2_sb[:, t5 * 32:(t5 + 1) * 32],
                                     mp[:, t5:t5 + CONV2_LEN],
                                     start=(t5 == 0), stop=(t5 == 4))
                nc.scalar.activation(
                    y2all[:, g * CONV2_LEN:(g + 1) * CONV2_LEN], y2p[:],
                    RELU, bias=b2_sb[:, 0:1])

            # ---- GCN layers 1..4, with tail interleaved into layer 4 ----
            _mark(nc, "layers")
            h_prev = h0
            for l in range(NLAYERS):
                _mark(nc, f"layer{l}")
                h_next = hp.tile([128, NLOC], F32, tag="h")
                h_layers.append(h_next)
                for g in range(GPC):
                    layer_graph(l, g, h_prev, h_next)
                    if l == NLAYERS - 1:
                        tail_t1(g)
                        if g >= 1:
                            tail_t2(g - 1)
                        if g >= 2:
                            tail_t3(g - 2)
                        if g >= 3:
                            tail_t3b(g - 3)
                h_prev = h_next
            if debug:
                for l in range(NLAYERS):
                    nc.sync.dma_start(DBG_H[l, :, :], h_layers[l][:])
            tail_t2(GPC - 1)
            for g in range(GPC - 2, GPC):
                tail_t3(g)
            for g in range(GPC - 3, GPC):
                tail_t3b(g)
            if debug:
                nc.sync.dma_start(DBG_Y2[:], y2all[:])

            _mark(nc, "densetail")
            # ---- core-level dense tail (batched over the 8 graphs) ----
            h1p = ps1.tile([32, GPC], F32, tag="ps128")
            y2v = y2all[:].rearrange("p (g t) -> p g t", t=CONV2_LEN)
            for t5 in range(CONV2_LEN):
                nc.tensor.matmul(h1p[:], d1_sb[:, t5 * 32:(t5 + 1) * 32],
                                 y2v[:, :, t5:t5 + 1],
                                 start=(t5 == 0), stop=(t5 == CONV2_LEN - 1))
            h1s = smp.tile([32, GPC], F32, tag="h1s")
            nc.scalar.activation(h1s[:], h1p[:], RELU, bias=bd1_sb[:, 0:1])
            lgp = ps1.tile([2, GPC], F32, tag="ps128")
            nc.tensor.matmul(lgp[:], d2_sb[:], h1s[:], start=True, stop=True)
            lg = smp.tile([2, GPC], F32, tag="lg")
            nc.vector.tensor_scalar(out=lg[:], in0=lgp[:],
                                    scalar1=bd2_sb[:, 0:1], scalar2=None,
                                    op0=ADD)
            dfp = ps1.tile([2, GPC], F32, tag="ps128")
            nc.tensor.matmul(dfp[:], diff_sb[:], lg[:], start=True, stop=True)
            pr = smp.tile([2, GPC], F32, tag="pr")
            nc.scalar.activation(pr[:], dfp[:], SIGM)
            nc.sync.dma_start(OUT[:], pr[:])

    nc.compile()
    return nc


def _get_nc(dt_key, with_bias, debug):
    key = (dt_key, with_bias, debug)
    if key not in _NC_CACHE:
        mode = "fast" if dt_key == "f32r" else "comp"
        _NC_CACHE[key] = _build(mode, with_bias, debug)
    return _NC_CACHE[key]


def prepare_host(inputs, dt_key):
    """All host-side index preprocessing + per-core input maps."""
    comp = dt_key != "f32r"
    x = np.asarray(inputs["x"]).astype(np.int64)
    edge_index = np.asarray(inputs["edge_index"]).astype(np.int64)
    emb = np.ascontiguousarray(np.asarray(inputs["emb"], dtype=np.float32))
    W_convs = np.asarray(inputs["W_convs"], dtype=np.float32)
    b_convs = np.asarray(inputs["b_convs"], dtype=np.float32)
    W_last = np.asarray(inputs["W_last"], dtype=np.float32)
    b_last = np.asarray(inputs["b_last"], dtype=np.float32)
    conv1_w = np.asarray(inputs["conv1_w"], dtype=np.float32)
    conv1_b = np.asarray(inputs["conv1_b"], dtype=np.float32)
    conv2_w = np.asarray(inputs["conv2_w"], dtype=np.float32)
    conv2_b = np.asarray(inputs["conv2_b"], dtype=np.float32)
    d1_w = np.asarray(inputs["d1_w"], dtype=np.float32)
    d1_b = np.asarray(inputs["d1_b"], dtype=np.float32)
    d2_w = np.asarray(inputs["d2_w"], dtype=np.float32)
    d2_b = np.asarray(inputs["d2_b"], dtype=np.float32)

    src, dst = edge_index[0], edge_index[1]
    deg = (np.bincount(src, minlength=N_TOTAL) + 1).astype(np.float32)
    invdeg = (np.float32(1.0) / deg).astype(np.float32)
    gid = dst >> 9
    flat = (gid * NPG + (dst & 511)) * NPG + (src & 511)
    A = np.bincount(flat, minlength=NUM_GRAPHS * NPG * NPG)
    A = A.astype(np.float32).reshape(NUM_GRAPHS, NPG, NPG)
    idx = np.arange(NPG)
    A[:, idx, idx] += 1.0
    # A^T chunks: ATD[g, c, i, d] = A[g, d, c*128+i]
    AT = np.ascontiguousarray(A.transpose(0, 2, 1)).reshape(
        NUM_GRAPHS, 4, 128, NPG)
    if comp:
        AT = AT.astype(ml_dtypes.bfloat16)  # small ints: exact

    h0 = emb[x]  # [N, 128]
    with_bias = bool(np.any(b_convs) or np.any(b_last))

    w1 = np.ascontiguousarray(conv1_w[:, 0, :].T)  # [513, 16]
    shared = {
        "WC": np.ascontiguousarray(W_convs),
        "W5": np.ascontiguousarray(W_last),
        "IDN": np.eye(128, dtype=np.float32),
        "KI": np.ascontiguousarray(
            np.broadcast_to(np.arange(K, dtype=np.float32), (128, K))),
        "MJ": np.ascontiguousarray(
            (np.arange(NPG)[None, None, :]
             < (np.arange(4)[:, None, None] * 128
                + np.arange(128)[None, :, None])).astype(np.float32)),
        "W1A": np.ascontiguousarray(w1[:512].reshape(4, 128, 16)),
        "W1BR": np.ascontiguousarray(w1[512:513]),        # [1, 16]
        "B1R": np.ascontiguousarray(conv1_b.reshape(1, 16)),
        "ONER": np.ones((1, 128), dtype=np.float32),
        "W2T": np.ascontiguousarray(conv2_w.transpose(2, 1, 0)),
        "B2": np.ascontiguousarray(conv2_b.reshape(32, 1)),
        "D1R": np.ascontiguousarray(d1_w.reshape(DD, CONV2_LEN * 32)
                                    .astype(np.float32)),
        "BD1": np.ascontiguousarray(d1_b.reshape(32, 1)),
        "D2": np.ascontiguousarray(d2_w),
        "BD2": np.ascontiguousarray(d2_b.reshape(2, 1)),
        "DIFF": np.array([[1.0, -1.0], [-1.0, 1.0]], dtype=np.float32),
    }
    if with_bias:
        shared["ONE"] = np.ones((1, 128), dtype=np.float32)
        shared["BCV"] = np.ascontiguousarray(
            b_convs.reshape(NLAYERS, 1, 128))
        shared["B5V"] = np.ascontiguousarray(b_last.reshape(1, 1))

    invg = invdeg.reshape(NUM_GRAPHS, NPG)
    in_maps = []
    for c in range(NCORES):
        h0c = np.ascontiguousarray(h0[c * NLOC:(c + 1) * NLOC].T)
        iv = invg[c * GPC:(c + 1) * GPC]                  # [GPC, 512]
        m = dict(shared)
        m["H0T"] = h0c
        m["ATD"] = np.ascontiguousarray(AT[c * GPC:(c + 1) * GPC])
        m["INVR"] = np.ascontiguousarray(iv.reshape(GPC, 1, NPG))
        m["INV4"] = np.ascontiguousarray(
            iv.reshape(GPC, 4, 128).transpose(0, 2, 1))  # [GPC, 128, 4]
        in_maps.append(m)
    return in_maps, with_bias


def run(inputs, dt_key="f32", debug=False, **spmd_kwargs):
    in_maps, with_bias = prepare_host(inputs, dt_key)
    nc = _get_nc(dt_key, with_bias, debug)
    res = run_bass_kernel_spmd(nc, in_maps, core_ids=list(range(NCORES)),
                               **spmd_kwargs)
    out = np.empty((NUM_GRAPHS, 2), dtype=np.float32)
    for c in range(NCORES):
        out[c * GPC:(c + 1) * GPC, :] = res.results[c]["OUT"].T
    return out, res


def kernel(**inputs):
    out, _ = run(inputs, dt_key="f32")
    return out


# revision 23
# speedup vs baseline: 1.4717x; 1.1077x over previous
"""DGCNN (gnn_message_passing) Trainium2 Bass kernel, v2.

Strategy (data-parallel over graphs, 8 graphs per NeuronCore):
  - Host builds, per graph, the INTEGER operator A^T where A = adj-multiplicity
    + I (entries are small ints, exactly representable in bf16), shipped as
    4 chunks of [128, 512] bf16. The degree normalization inv = 1/deg is
    applied on-device AFTER the matmul (elementwise, exact fp32), so the
    aggregation matmul never rounds the operator.
  - Each GCN layer, per graph:
      lin  = h @ W          (fp32 matmuls, exact, node-major chunks)
      hi   = bf16(lin); lo = bf16(lin - hi)   (ACT copy + DVE subtract)
      u    = A^T-chunks x (hi | lo) -> PSUM [128f, 512d]  (8 bf16 matmuls at
             1 cyc/row -- 2x faster than one fp32 matmul, fp32-exact result)
      msg  = u * invb       (Pool elementwise)
      h'   = tanh(msg)      (ACT)
    This reproduces the reference to ~3e-7 (verified in numpy: zero top-k
    rank flips).
  - Layer 5 (h5): fp32/bf16-comp matvecs against the same A^T chunks,
    inv scale + tanh node-major, then transpose -> row form -> broadcast.
  - Ranks: exact stable rank[i] = #{v>v_i} + #{j<i: v==v_i} via DVE(is_gt)
    + Pool(is_eq*mask) passes; one-hot selection matrix PT from ranks.
  - Head (conv1-first): y1[o,d] = relu(W1 . xcat[:,d]) computed for ALL 512
    columns per graph via 4 f32r matmuls (free=512) + DVE rank-1 h5 update;
    then the CHEAP [16,512] tensor is transposed (4 tiny PE transposes) and
    64 columns selected by one-hot matmuls -- this replaces transposing all
    4 [128,512] h matrices per graph of the old design.
  - maxpool/conv2/dense head as small fp32 matmuls; final 2-class softmax
    via sigmoid of logit differences.

Modes: dt_key "f32" = compensated (default, ~3e-7 rel err);
       "f32r" = 1-term f32r aggregation (faster, ~1e-2 rel err).

Self-contained: hardcodes all shapes; no reads of /root/problem files.
"""

import sys

if "/opt/trn_rl_repo" not in sys.path:
    sys.path.insert(0, "/opt/trn_rl_repo")

import ml_dtypes
import numpy as np

import concourse.bacc as bacc
import concourse.mybir as mybir
import concourse.tile as tile
from concourse.bass_utils import run_bass_kernel_spmd

F32 = mybir.dt.float32
F32R = mybir.dt.float32r
BF16 = mybir.dt.bfloat16

NUM_GRAPHS = 64
NPG = 512  # nodes per graph
N_TOTAL = NUM_GRAPHS * NPG
EMB = 128
DIMF = 128
NLAYERS = 4
K = 64
NCORES = 8
GPC = NUM_GRAPHS // NCORES  # graphs per core = 8
NLOC = GPC * NPG  # local nodes = 4096
LATENT = NLAYERS * DIMF + 1  # 513
DD = (K - 2) // 2 + 1  # 32
CONV2_LEN = DD - 5 + 1  # 28

_NC_CACHE = {}
SECTION_MARKS = []  # (label, id_at_boundary) for profiling analysis


def _mark(nc, label):
    SECTION_MARKS.append((label, nc.next_id()))


def _build(mode, with_bias, debug):
    """Trace + compile the per-core Bass program (same on all 8 cores).

    mode: "comp" (bf16 A + hi/lo compensated agg, near-exact) or
          "fast" (f32r A + single rounded agg).
    """
    comp = mode == "comp"
    DT_A = BF16 if comp else F32R

    nc = bacc.Bacc("TRN2", target_bir_lowering=False, debug=False,
                   num_devices=NCORES)

    # ---- per-core DRAM I/O ----
    H0T = nc.dram_tensor("H0T", [128, NLOC], F32, kind="ExternalInput")
    ATD = nc.dram_tensor("ATD", [GPC, 4, 128, NPG], DT_A,
                         kind="ExternalInput")
    WC = nc.dram_tensor("WC", [NLAYERS, 128, 128], F32, kind="ExternalInput")
    W5 = nc.dram_tensor("W5", [128, 1], F32, kind="ExternalInput")
    INVR = nc.dram_tensor("INVR", [GPC, 1, NPG], F32, kind="ExternalInput")
    INV4 = nc.dram_tensor("INV4", [GPC, 128, 4], F32, kind="ExternalInput")
    IDN = nc.dram_tensor("IDN", [128, 128], F32, kind="ExternalInput")
    KI = nc.dram_tensor("KI", [128, K], F32, kind="ExternalInput")
    MJ = nc.dram_tensor("MJ", [4, 128, NPG], F32, kind="ExternalInput")
    W1A = nc.dram_tensor("W1A", [4, 128, 16], F32, kind="ExternalInput")
    W1BR = nc.dram_tensor("W1BR", [1, 16], F32, kind="ExternalInput")
    B1R = nc.dram_tensor("B1R", [1, 16], F32, kind="ExternalInput")
    ONER = nc.dram_tensor("ONER", [1, 128], F32, kind="ExternalInput")
    W2T = nc.dram_tensor("W2T", [5, 16, 32], F32, kind="ExternalInput")
    B2 = nc.dram_tensor("B2", [32, 1], F32, kind="ExternalInput")
    D1R = nc.dram_tensor("D1R", [32, CONV2_LEN * 32], F32,
                         kind="ExternalInput")
    BD1 = nc.dram_tensor("BD1", [32, 1], F32, kind="ExternalInput")
    D2 = nc.dram_tensor("D2", [32, 2], F32, kind="ExternalInput")
    BD2 = nc.dram_tensor("BD2", [2, 1], F32, kind="ExternalInput")
    DIFF = nc.dram_tensor("DIFF", [2, 2], F32, kind="ExternalInput")
    if with_bias:
        ONE = nc.dram_tensor("ONE", [1, 128], F32, kind="ExternalInput")
        BCV = nc.dram_tensor("BCV", [NLAYERS, 1, 128], F32,
                             kind="ExternalInput")
        B5V = nc.dram_tensor("B5V", [1, 1], F32, kind="ExternalInput")
    OUT = nc.dram_tensor("OUT", [2, GPC], F32, kind="ExternalOutput")
    if debug:
        DBG_H = nc.dram_tensor("DBG_H", [NLAYERS, 128, NLOC], F32,
                               kind="ExternalOutput")
        DBG_H5 = nc.dram_tensor("DBG_H5", [GPC, 1, NPG], F32,
                                kind="ExternalOutput")
        DBG_RANK = nc.dram_tensor("DBG_RANK", [GPC, 128, 4], F32,
                                  kind="ExternalOutput")
        DBG_Y1 = nc.dram_tensor("DBG_Y1", [GPC, 128, 4 * 16], F32,
                                kind="ExternalOutput")
        DBG_SEL = nc.dram_tensor("DBG_SEL", [GPC, 16, K], F32,
                                 kind="ExternalOutput")
        DBG_Y2 = nc.dram_tensor("DBG_Y2", [32, GPC * CONV2_LEN], F32,
                                kind="ExternalOutput")

    TANH = mybir.ActivationFunctionType.Tanh
    RELU = mybir.ActivationFunctionType.Relu
    SIGM = mybir.ActivationFunctionType.Sigmoid
    ADD = mybir.AluOpType.add
    SUB = mybir.AluOpType.subtract
    MULT = mybir.AluOpType.mult
    MAX = mybir.AluOpType.max
    IS_GT = mybir.AluOpType.is_gt
    IS_EQ = mybir.AluOpType.is_equal

    with tile.TileContext(nc) as tc:
        with (
            tc.tile_pool(name="const", bufs=1) as cp,
            tc.tile_pool(name="hs", bufs=4) as hp,
            tc.tile_pool(name="at", bufs=1) as atp,
            tc.tile_pool(name="iv", bufs=1) as ivp,
            tc.tile_pool(name="hi", bufs=3) as hip,
            tc.tile_pool(name="lo", bufs=3) as lop,
            tc.tile_pool(name="sc5", bufs=3) as sc5p,
            tc.tile_pool(name="vbp", bufs=4) as vbp,
            tc.tile_pool(name="ptp", bufs=4) as ptp,
            tc.tile_pool(name="sm", bufs=6) as smp,
            tc.tile_pool(name="scr", bufs=3) as scp,
            tc.tile_pool(name="y1", bufs=4) as y1p_pool,
            tc.tile_pool(name="pslp", bufs=2, space="PSUM") as lpp,
            tc.tile_pool(name="ps512", bufs=2, space="PSUM") as ps5,
            tc.tile_pool(name="psrow", bufs=1, space="PSUM") as psr,
            tc.tile_pool(name="ps128", bufs=3, space="PSUM") as ps1,
        ):
            # ---- constant / weight loads (first-needed first) ----
            wc_sb = cp.tile([128, NLAYERS * 128], F32, tag="wc")
            h0 = hp.tile([128, NLOC], F32, tag="h")
            at_sb = []
            for g in range(GPC):
                at_t = atp.tile([128, 4 * NPG], DT_A, tag=f"at{g}",
                                name=f"at_t{g}")
                at_sb.append(at_t)

            def load_at(g):
                for c in range(4):
                    nc.sync.dma_start(at_sb[g][:, c * NPG:(c + 1) * NPG],
                                      ATD[g, c, :, :])

            invb = []
            for g in range(GPC):
                t = ivp.tile([128, NPG], F32, tag=f"invb{g}",
                             name=f"invb{g}")
                invb.append(t)

            def load_inv(g):
                ir = smp.tile([1, NPG], F32, tag="inr")
                nc.sync.dma_start(ir[0:1, :], INVR[g, :, :])
                nc.gpsimd.partition_broadcast(invb[g][:], ir[0:1, :])

            # minimal first-matmul working set first: wc layer-0 slice, the
            # first h0 sliver, then graph-0's A chunks + inv; remaining
            # weights interleave behind
            nc.sync.dma_start(wc_sb[:, 0:128], WC[0, :, :])
            for s in range(4):
                nc.sync.dma_start(h0[:, s * 128:(s + 1) * 128],
                                  H0T[:, s * 128:(s + 1) * 128])
            load_at(0)
            load_inv(0)
            for l in range(1, NLAYERS):
                nc.sync.dma_start(wc_sb[:, l * 128:(l + 1) * 128], WC[l, :, :])
            for c in range(1, 4):
                nc.sync.dma_start(h0[:, c * 512:(c + 1) * 512],
                                  H0T[:, c * 512:(c + 1) * 512])
                load_at(c)
                load_inv(c)
            for c in range(4, 8):
                nc.sync.dma_start(h0[:, c * 512:(c + 1) * 512],
                                  H0T[:, c * 512:(c + 1) * 512])
            for g in range(4, GPC):
                load_at(g)
                load_inv(g)
            w5_sb = cp.tile([128, 1], F32, tag="w5")
            nc.sync.dma_start(w5_sb[:], W5[:])
            inv4_sb = cp.tile([128, 4 * GPC], F32, tag="inv4")
            for g in range(GPC):
                nc.sync.dma_start(inv4_sb[:, g * 4:(g + 1) * 4],
                                  INV4[g, :, :])
            id_sb = cp.tile([128, 128], F32, tag="idn")
            nc.sync.dma_start(id_sb[:], IDN[:])
            ki_sb = cp.tile([128, K], F32, tag="ki")
            nc.sync.dma_start(ki_sb[:], KI[:])
            mj_sb = cp.tile([128, 4 * NPG], F32, tag="mj")
            for c in range(4):
                nc.sync.dma_start(mj_sb[:, c * NPG:(c + 1) * NPG], MJ[c, :, :])
            w1a_sb = cp.tile([128, 64], F32, tag="w1a")
            for c in range(4):
                nc.sync.dma_start(w1a_sb[:, c * 16:(c + 1) * 16], W1A[c, :, :])
            b1r_sb = cp.tile([1, 16], F32, tag="b1r")
            nc.sync.dma_start(b1r_sb[:], B1R[:])
            oner_sb = cp.tile([1, 128], F32, tag="oner")
            nc.sync.dma_start(oner_sb[:], ONER[:])
            w1br_sb = cp.tile([1, 16], F32, tag="w1br")
            nc.sync.dma_start(w1br_sb[:], W1BR[:])
            w1bb = cp.tile([128, 16], F32, tag="w1bb")
            nc.gpsimd.partition_broadcast(w1bb[:], w1br_sb[0:1, :])
            w2_sb = cp.tile([16, 160], F32, tag="w2t")
            for t5 in range(5):
                nc.sync.dma_start(w2_sb[:, t5 * 32:(t5 + 1) * 32],
                                  W2T[t5, :, :])
            b2_sb = cp.tile([32, 1], F32, tag="b2")
            nc.sync.dma_start(b2_sb[:], B2[:])
            d1_sb = cp.tile([32, CONV2_LEN * 32], F32, tag="d1r")
            nc.sync.dma_start(d1_sb[:], D1R[:])
            bd1_sb = cp.tile([32, 1], F32, tag="bd1")
            nc.sync.dma_start(bd1_sb[:], BD1[:])
            d2_sb = cp.tile([32, 2], F32, tag="d2")
            nc.sync.dma_start(d2_sb[:], D2[:])
            bd2_sb = cp.tile([2, 1], F32, tag="bd2")
            nc.sync.dma_start(bd2_sb[:], BD2[:])
            diff_sb = cp.tile([2, 2], F32, tag="diff")
            nc.sync.dma_start(diff_sb[:], DIFF[:])
            if with_bias:
                one_sb = cp.tile([1, 128], F32, tag="one")
                nc.sync.dma_start(one_sb[:], ONE[:])
                bcv_sb = []
                for l in range(NLAYERS):
                    t = cp.tile([1, 128], F32, tag=f"bcv{l}")
                    nc.sync.dma_start(t[:], BCV[l, :, :])
                    bcv_sb.append(t)
                b5v_sb = cp.tile([1, 1], F32, tag="b5v")
                nc.sync.dma_start(b5v_sb[:], B5V[:])
            y2all = cp.tile([32, GPC * CONV2_LEN], F32, tag="y2all")

            h_layers = []
            splits = {}

            def emit_lin(l, g, h_prev):
                """4 fp32 matmuls into one [128, 512] PSUM (node-chunk-major
                columns), then ONE hi/lo bf16 split (ACT + DVE)."""
                lp = lpp.tile([128, NPG], F32, tag="lp")
                for cc in range(4):
                    ch = 4 * g + cc
                    nc.tensor.matmul(
                        lp[:, cc * 128:(cc + 1) * 128],
                        h_prev[:, ch * 128:(ch + 1) * 128],
                        wc_sb[:, l * 128:(l + 1) * 128],
                        start=True, stop=not with_bias)
                    if with_bias:
                        nc.tensor.matmul(lp[:, cc * 128:(cc + 1) * 128],
                                         one_sb[:], bcv_sb[l][:],
                                         start=False, stop=True)
                if comp:
                    hi = hip.tile([128, NPG], BF16, tag="hi")
                    nc.scalar.copy(hi[:], lp[:])
                    lo = lop.tile([128, NPG], BF16, tag="lo")
                    nc.vector.tensor_tensor(out=lo[:], in0=lp[:],
                                            in1=hi[:], op=SUB)
                    splits[(l, g)] = (hi, lo)
                else:
                    lr = hip.tile([128, NPG], F32R, tag="hi")
                    nc.vector.tensor_copy(lr[:], lp[:])
                    splits[(l, g)] = (lr,)

            def emit_agg(l, g):
                """bf16 agg matmuls (exact integer A x hi/lo), inv row-scale
                (DVE), tanh (ACT) -> h feature-major."""
                arrs = splits.pop((l, g))
                sp = ps5.tile([128, NPG], F32, tag="ps512")
                nmm = 4 * len(arrs)
                i = 0
                for arr in arrs:
                    for cc in range(4):
                        nc.tensor.matmul(
                            sp[:], arr[:, cc * 128:(cc + 1) * 128],
                            at_sb[g][:, cc * NPG:(cc + 1) * NPG],
                            start=(i == 0), stop=(i == nmm - 1))
                        i += 1
                sc5 = sc5p.tile([128, NPG], F32, tag="sc5")
                nc.vector.tensor_tensor(out=sc5[:], in0=sp[:],
                                        in1=invb[g][:], op=MULT)
                nc.scalar.activation(
                    h_layers[l][:, g * NPG:(g + 1) * NPG], sc5[:], TANH)

            # ---- tail stages, software-pipelined across graphs ----
            vcols, vbs, ptts, y1ts, sels = {}, {}, {}, {}, {}

            def tail_t1(g):
                """h5 for graph g: fp32 matvecs for lin5, bf16-comp row-form
                aggregation (free=512 chains), inv + tanh on the row, then
                partition broadcast + node-major vcol via tiny transposes."""
                _mark(nc, f"t1_g{g}")
                h4 = h_layers[NLAYERS - 1]
                l5p = ps1.tile([128, 4], F32, tag="ps128")
                for cc in range(4):
                    ch = 4 * g + cc
                    nc.tensor.matmul(
                        l5p[:, cc:cc + 1],
                        h4[:, ch * 128:(ch + 1) * 128], w5_sb[:],
                        start=True, stop=not with_bias)
                    if with_bias:
                        nc.tensor.matmul(l5p[:, cc:cc + 1], one_sb[:],
                                         b5v_sb[:], start=False, stop=True)
                m5r = psr.tile([1, NPG], F32, tag="psrow")
                if comp:
                    hl5 = smp.tile([128, 8], BF16, tag="hl5")
                    nc.scalar.copy(hl5[:, 0:4], l5p[:])
                    nc.vector.tensor_tensor(out=hl5[:, 4:8], in0=l5p[:],
                                            in1=hl5[:, 0:4], op=SUB)
                    cols = [0, 1, 2, 3, 4, 5, 6, 7]
                else:
                    hl5 = smp.tile([128, 4], F32R, tag="hl5")
                    nc.vector.tensor_copy(hl5[:], l5p[:])
                    cols = [0, 1, 2, 3]
                for i, col in enumerate(cols):
                    sc = col % 4
                    nc.tensor.matmul(
                        m5r[:], hl5[:, col:col + 1],
                        at_sb[g][:, sc * NPG:(sc + 1) * NPG],
                        start=(i == 0), stop=(i == len(cols) - 1))
                m5v = smp.tile([1, NPG], F32, tag="m5v")
                nc.vector.tensor_tensor(out=m5v[:], in0=m5r[:],
                                        in1=invb[g][0:1, :], op=MULT)
                h5r = smp.tile([1, NPG], F32, tag="h5r")
                nc.scalar.activation(h5r[:], m5v[:], TANH)
                vb = vbp.tile([128, NPG], F32, tag="vb")
                nc.gpsimd.partition_broadcast(vb[:], h5r[0:1, :])
                # node-major vcol [128, 4] via 4 tiny PE transposes
                vpall = ps1.tile([128, 4], F32, tag="ps128")
                for cc in range(4):
                    nc.tensor.transpose(vpall[:, cc:cc + 1],
                                        h5r[0:1, cc * 128:(cc + 1) * 128],
                                        id_sb[0:1, 0:1])
                vcol = smp.tile([128, 4], F32, tag="vcol")
                nc.vector.tensor_copy(vcol[:], vpall[:])
                vcols[g] = vcol
                vbs[g] = vb
                if debug:
                    nc.sync.dma_start(DBG_H5[g, :, :], h5r[:])

            def tail_t2(g):
                """Exact stable ranks + one-hot selection matrix."""
                _mark(nc, f"t2_g{g}")
                vb, vcol = vbs[g], vcols[g]
                rank = smp.tile([128, 4], F32, tag="rank")
                for cc in range(4):
                    t1s = scp.tile([128, NPG], F32, tag="tt")
                    ra = smp.tile([128, 2], F32, tag="ra")
                    nc.vector.tensor_scalar(
                        out=t1s[:], in0=vb[:], scalar1=vcol[:, cc:cc + 1],
                        scalar2=None, op0=IS_GT, op1=ADD,
                        accum_out=ra[:, 0:1])
                    t2s = scp.tile([128, NPG], F32, tag="tt")
                    nc.vector.scalar_tensor_tensor(
                        out=t2s[:], in0=vb[:], scalar=vcol[:, cc:cc + 1],
                        in1=mj_sb[:, cc * NPG:(cc + 1) * NPG],
                        op0=IS_EQ, op1=MULT, accum_out=ra[:, 1:2])
                    nc.vector.tensor_tensor(
                        out=rank[:, cc:cc + 1], in0=ra[:, 0:1],
                        in1=ra[:, 1:2], op=ADD)
                if debug:
                    nc.sync.dma_start(DBG_RANK[g, :, :], rank[:])
                ptt = ptp.tile([128, 4 * K], F32, tag="pt")
                for cc in range(4):
                    nc.vector.tensor_scalar(
                        out=ptt[:, cc * K:(cc + 1) * K], in0=ki_sb[:],
                        scalar1=rank[:, cc:cc + 1], scalar2=None, op0=IS_EQ)
                ptts[g] = ptt

            def tail_t3(g):
                """conv1-first, node-major: y1T[d,o] = relu(xcat[:,d].W1 + b1)
                for all 512 nodes via fp32 matmuls with h chunks stationary
                (free=16), h5 rank-1 term via per-partition DVE scalar op,
                then one-hot row selection -> sel [16, K]. All exact fp32."""
                _mark(nc, f"t3_g{g}")
                vcol, ptt = vcols[g], ptts[g]
                y1t = y1p_pool.tile([128, 4 * 16], F32, tag="y1t")
                for cc in range(4):
                    ch = 4 * g + cc
                    p = ps1.tile([128, 16], F32, tag="ps128")
                    for l in range(NLAYERS):
                        nc.tensor.matmul(
                            p[:], h_layers[l][:, ch * 128:(ch + 1) * 128],
                            w1a_sb[:, l * 16:(l + 1) * 16],
                            start=(l == 0), stop=False)
                    nc.tensor.matmul(p[:], oner_sb[:], b1r_sb[:],
                                     start=False, stop=True)
                    # pre-relu y1 with the h5 rank-1 term, straight to SBUF
                    nc.vector.scalar_tensor_tensor(
                        out=y1t[:, cc * 16:(cc + 1) * 16], in0=w1bb[:],
                        scalar=vcol[:, cc:cc + 1],
                        in1=p[:], op0=MULT, op1=ADD)
                if debug:
                    nc.sync.dma_start(DBG_Y1[g, :, :], y1t[:])
                # select 64 rows (nodes) via one-hot matmuls; relu commutes
                # with one-hot selection so it's folded into the PSUM drain
                selp = ps1.tile([16, K], F32, tag="ps128")
                for cc in range(4):
                    nc.tensor.matmul(selp[:], y1t[:, cc * 16:(cc + 1) * 16],
                                     ptt[:, cc * K:(cc + 1) * K],
                                     start=(cc == 0), stop=(cc == 3))
                sel = smp.tile([16, K], F32, tag="sel")
                nc.scalar.activation(sel[:], selp[:], RELU)
                sels[g] = sel
                if debug:
                    nc.sync.dma_start(DBG_SEL[g, :, :], sel[:])

            def tail_t3b(g):
                """maxpool(2) -> conv2 -> relu into y2all."""
                _mark(nc, f"t3b_g{g}")
                sel = sels[g]
                mp = smp.tile([16, K // 2], F32, tag="mp")
                selv = sel[:].rearrange("p (a b) -> p a b", b=2)
                nc.vector.tensor_tensor(out=mp[:], in0=selv[:, :, 0:1],
                                        in1=selv[:, :, 1:2], op=MAX)
                y2p = ps1.tile([32, CONV2_LEN], F32, tag="ps128")
                for t5 in range(5):
                    nc.tensor.matmul(y2p[:], w2_sb[:, t5 * 32:(t5 + 1) * 32],
                                     mp[:, t5:t5 + CONV2_LEN],
                                     start=(t5 == 0), stop=(t5 == 4))
                nc.scalar.activation(
                    y2all[:, g * CONV2_LEN:(g + 1) * CONV2_LEN], y2p[:],
                    RELU, bias=b2_sb[:, 0:1])

            # ---- GCN layers 1..4, software-pipelined emission: the agg of
            # slot i-D is emitted after the lin of slot i so the PE stream
            # never head-of-line blocks on the hi/lo split. Tail stages for
            # graph g are emitted right after layer-4's agg(g), themselves
            # pipelined across graphs.
            _mark(nc, "layers")
            for l in range(NLAYERS):
                h_layers.append(hp.tile([128, NLOC], F32, tag="h",
                                        name=f"h{l + 1}"))

            def tail_after_agg(l, g):
                if l != NLAYERS - 1:
                    return
                tail_t1(g)
                if g >= 1:
                    tail_t2(g - 1)
                if g >= 2:
                    tail_t3(g - 2)
                if g >= 3:
                    tail_t3b(g - 3)

            slots = [(l, g) for l in range(NLAYERS) for g in range(GPC)]
            D = 1  # agg deferral distance (pipeline depth)
            for i, (l, g) in enumerate(slots):
                if g == 0:
                    _mark(nc, f"layer{l}")
                emit_lin(l, g, h0 if l == 0 else h_layers[l - 1])
                if i >= D:
                    la, ga = slots[i - D]
                    emit_agg(la, ga)
                    tail_after_agg(la, ga)
            for j in range(len(slots) - D, len(slots)):
                la, ga = slots[j]
                emit_agg(la, ga)
                tail_after_agg(la, ga)
            if debug:
                for l in range(NLAYERS):
                    nc.sync.dma_start(DBG_H[l, :, :], h_layers[l][:])
            tail_t2(GPC - 1)
            for g in range(GPC - 2, GPC):
                tail_t3(g)
            for g in range(GPC - 3, GPC):
                tail_t3b(g)
            if debug:
                nc.sync.dma_start(DBG_Y2[:], y2all[:])

            _mark(nc, "densetail")
            # ---- core-level dense tail (batched over the 8 graphs) ----
            h1p = ps1.tile([32, GPC], F32, tag="ps128")
            y2v = y2all[:].rearrange("p (g t) -> p g t", t=CONV2_LEN)
            for t5 in range(CONV2_LEN):
                nc.tensor.matmul(h1p[:], d1_sb[:, t5 * 32:(t5 + 1) * 32],
                                 y2v[:, :, t5:t5 + 1],
                                 start=(t5 == 0), stop=(t5 == CONV2_LEN - 1))
            h1s = smp.tile([32, GPC], F32, tag="h1s")
            nc.scalar.activation(h1s[:], h1p[:], RELU, bias=bd1_sb[:, 0:1])
            lgp = ps1.tile([2, GPC], F32, tag="ps128")
            nc.tensor.matmul(lgp[:], d2_sb[:], h1s[:], start=True, stop=True)
            lg = smp.tile([2, GPC], F32, tag="lg")
            nc.vector.tensor_scalar(out=lg[:], in0=lgp[:],
                                    scalar1=bd2_sb[:, 0:1], scalar2=None,
                                    op0=ADD)
            dfp = ps1.tile([2, GPC], F32, tag="ps128")
            nc.tensor.matmul(dfp[:], diff_sb[:], lg[:], start=True, stop=True)
            pr = smp.tile([2, GPC], F32, tag="pr")
            nc.scalar.activation(pr[:], dfp[:], SIGM)
            nc.sync.dma_start(OUT[:], pr[:])

    nc.compile()
    return nc


def _get_nc(dt_key, with_bias, debug):
    key = (dt_key, with_bias, debug)
    if key not in _NC_CACHE:
        mode = "fast" if dt_key == "f32r" else "comp"
        _NC_CACHE[key] = _build(mode, with_bias, debug)
    return _NC_CACHE[key]


def prepare_host(inputs, dt_key):
    """All host-side index preprocessing + per-core input maps."""
    comp = dt_key != "f32r"
    x = np.asarray(inputs["x"]).astype(np.int64)
    edge_index = np.asarray(inputs["edge_index"]).astype(np.int64)
    emb = np.ascontiguousarray(np.asarray(inputs["emb"], dtype=np.float32))
    W_convs = np.asarray(inputs["W_convs"], dtype=np.float32)
    b_convs = np.asarray(inputs["b_convs"], dtype=np.float32)
    W_last = np.asarray(inputs["W_last"], dtype=np.float32)
    b_last = np.asarray(inputs["b_last"], dtype=np.float32)
    conv1_w = np.asarray(inputs["conv1_w"], dtype=np.float32)
    conv1_b = np.asarray(inputs["conv1_b"], dtype=np.float32)
    conv2_w = np.asarray(inputs["conv2_w"], dtype=np.float32)
    conv2_b = np.asarray(inputs["conv2_b"], dtype=np.float32)
    d1_w = np.asarray(inputs["d1_w"], dtype=np.float32)
    d1_b = np.asarray(inputs["d1_b"], dtype=np.float32)
    d2_w = np.asarray(inputs["d2_w"], dtype=np.float32)
    d2_b = np.asarray(inputs["d2_b"], dtype=np.float32)

    src, dst = edge_index[0], edge_index[1]
    deg = (np.bincount(src, minlength=N_TOTAL) + 1).astype(np.float32)
    invdeg = (np.float32(1.0) / deg).astype(np.float32)
    gid = dst >> 9
    flat = (gid * NPG + (dst & 511)) * NPG + (src & 511)
    A = np.bincount(flat, minlength=NUM_GRAPHS * NPG * NPG)
    A = A.astype(np.float32).reshape(NUM_GRAPHS, NPG, NPG)
    idx = np.arange(NPG)
    A[:, idx, idx] += 1.0
    # A^T chunks: ATD[g, c, i, d] = A[g, d, c*128+i]
    AT = np.ascontiguousarray(A.transpose(0, 2, 1)).reshape(
        NUM_GRAPHS, 4, 128, NPG)
    if comp:
        AT = AT.astype(ml_dtypes.bfloat16)  # small ints: exact

    h0 = emb[x]  # [N, 128]
    with_bias = bool(np.any(b_convs) or np.any(b_last))

    w1 = np.ascontiguousarray(conv1_w[:, 0, :].T)  # [513, 16]
    shared = {
        "WC": np.ascontiguousarray(W_convs),
        "W5": np.ascontiguousarray(W_last),
        "IDN": np.eye(128, dtype=np.float32),
        "KI": np.ascontiguousarray(
            np.broadcast_to(np.arange(K, dtype=np.float32), (128, K))),
        "MJ": np.ascontiguousarray(
            (np.arange(NPG)[None, None, :]
             < (np.arange(4)[:, None, None] * 128
                + np.arange(128)[None, :, None])).astype(np.float32)),
        "W1A": np.ascontiguousarray(w1[:512].reshape(4, 128, 16)),
        "W1BR": np.ascontiguousarray(w1[512:513]),        # [1, 16]
        "B1R": np.ascontiguousarray(conv1_b.reshape(1, 16)),
        "ONER": np.ones((1, 128), dtype=np.float32),
        "W2T": np.ascontiguousarray(conv2_w.transpose(2, 1, 0)),
        "B2": np.ascontiguousarray(conv2_b.reshape(32, 1)),
        "D1R": np.ascontiguousarray(d1_w.reshape(DD, CONV2_LEN * 32)
                                    .astype(np.float32)),
        "BD1": np.ascontiguousarray(d1_b.reshape(32, 1)),
        "D2": np.ascontiguousarray(d2_w),
        "BD2": np.ascontiguousarray(d2_b.reshape(2, 1)),
        "DIFF": np.array([[1.0, -1.0], [-1.0, 1.0]], dtype=np.float32),
    }
    if with_bias:
        shared["ONE"] = np.ones((1, 128), dtype=np.float32)
        shared["BCV"] = np.ascontiguousarray(
            b_convs.reshape(NLAYERS, 1, 128))
        shared["B5V"] = np.ascontiguousarray(b_last.reshape(1, 1))

    invg = invdeg.reshape(NUM_GRAPHS, NPG)
    in_maps = []
    for c in range(NCORES):
        h0c = np.ascontiguousarray(h0[c * NLOC:(c + 1) * NLOC].T)
        iv = invg[c * GPC:(c + 1) * GPC]                  # [GPC, 512]
        m = dict(shared)
        m["H0T"] = h0c
        m["ATD"] = np.ascontiguousarray(AT[c * GPC:(c + 1) * GPC])
        m["INVR"] = np.ascontiguousarray(iv.reshape(GPC, 1, NPG))
        m["INV4"] = np.ascontiguousarray(
            iv.reshape(GPC, 4, 128).transpose(0, 2, 1))  # [GPC, 128, 4]
        in_maps.append(m)
    return in_maps, with_bias


def run(inputs, dt_key="f32", debug=False, **spmd_kwargs):
    in_maps, with_bias = prepare_host(inputs, dt_key)
    nc = _get_nc(dt_key, with_bias, debug)
    res = run_bass_kernel_spmd(nc, in_maps, core_ids=list(range(NCORES)),
                               **spmd_kwargs)
    out = np.empty((NUM_GRAPHS, 2), dtype=np.float32)
    for c in range(NCORES):
        out[c * GPC:(c + 1) * GPC, :] = res.results[c]["OUT"].T
    return out, res


def kernel(**inputs):
    out, _ = run(inputs, dt_key="f32")
    return out


# revision 27
# speedup vs baseline: 1.5398x; 1.0463x over previous
"""DGCNN (gnn_message_passing) Trainium2 Bass kernel, v2.

Strategy (data-parallel over graphs, 8 graphs per NeuronCore):
  - Host builds, per graph, the INTEGER operator A^T where A = adj-multiplicity
    + I (entries are small ints, exactly representable in bf16), shipped as
    4 chunks of [128, 512] bf16. The degree normalization inv = 1/deg is
    applied on-device AFTER the matmul (elementwise, exact fp32), so the
    aggregation matmul never rounds the operator.
  - Each GCN layer, per graph:
      lin  = h @ W          (fp32 matmuls, exact, node-major chunks)
      hi   = bf16(lin); lo = bf16(lin - hi)   (ACT copy + DVE subtract)
      u    = A^T-chunks x (hi | lo) -> PSUM [128f, 512d]  (8 bf16 matmuls at
             1 cyc/row -- 2x faster than one fp32 matmul, fp32-exact result)
      msg  = u * invb       (Pool elementwise)
      h'   = tanh(msg)      (ACT)
    This reproduces the reference to ~3e-7 (verified in numpy: zero top-k
    rank flips).
  - Layer 5 (h5): fp32/bf16-comp matvecs against the same A^T chunks,
    inv scale + tanh node-major, then transpose -> row form -> broadcast.
  - Ranks: exact stable rank[i] = #{v>v_i} + #{j<i: v==v_i} via DVE(is_gt)
    + Pool(is_eq*mask) passes; one-hot selection matrix PT from ranks.
  - Head (conv1-first): y1[o,d] = relu(W1 . xcat[:,d]) computed for ALL 512
    columns per graph via 4 f32r matmuls (free=512) + DVE rank-1 h5 update;
    then the CHEAP [16,512] tensor is transposed (4 tiny PE transposes) and
    64 columns selected by one-hot matmuls -- this replaces transposing all
    4 [128,512] h matrices per graph of the old design.
  - maxpool/conv2/dense head as small fp32 matmuls; final 2-class softmax
    via sigmoid of logit differences.

Modes: dt_key "f32" = compensated (default, ~3e-7 rel err);
       "f32r" = 1-term f32r aggregation (faster, ~1e-2 rel err).

Self-contained: hardcodes all shapes; no reads of /root/problem files.
"""

import sys

if "/opt/trn_rl_repo" not in sys.path:
    sys.path.insert(0, "/opt/trn_rl_repo")

import ml_dtypes
import numpy as np

import concourse.bacc as bacc
import concourse.mybir as mybir
import concourse.tile as tile
from concourse.bass_utils import run_bass_kernel_spmd

F32 = mybir.dt.float32
F32R = mybir.dt.float32r
BF16 = mybir.dt.bfloat16

NUM_GRAPHS = 64
NPG = 512  # nodes per graph
N_TOTAL = NUM_GRAPHS * NPG
EMB = 128
DIMF = 128
NLAYERS = 4
K = 64
NCORES = 8
GPC = NUM_GRAPHS // NCORES  # graphs per core = 8
NLOC = GPC * NPG  # local nodes = 4096
LATENT = NLAYERS * DIMF + 1  # 513
DD = (K - 2) // 2 + 1  # 32
CONV2_LEN = DD - 5 + 1  # 28

_NC_CACHE = {}
SECTION_MARKS = []  # (label, id_at_boundary) for profiling analysis


def _mark(nc, label):
    SECTION_MARKS.append((label, nc.next_id()))


def _build(mode, with_bias, debug):
    """Trace + compile the per-core Bass program (same on all 8 cores).

    mode: "comp" (bf16 A + hi/lo compensated agg, near-exact) or
          "fast" (f32r A + single rounded agg).
    """
    comp = mode == "comp"
    DT_A = BF16 if comp else F32R

    nc = bacc.Bacc("TRN2", target_bir_lowering=False, debug=False,
                   num_devices=NCORES)

    # ---- per-core DRAM I/O ----
    H0T = nc.dram_tensor("H0T", [128, NLOC], F32, kind="ExternalInput")
    ATD = nc.dram_tensor("ATD", [GPC, 4, 128, NPG], DT_A,
                         kind="ExternalInput")
    WC = nc.dram_tensor("WC", [NLAYERS, 128, 128], F32, kind="ExternalInput")
    W5 = nc.dram_tensor("W5", [128, 1], F32, kind="ExternalInput")
    INVR = nc.dram_tensor("INVR", [GPC, 1, NPG], F32, kind="ExternalInput")
    INV4 = nc.dram_tensor("INV4", [GPC, 128, 4], F32, kind="ExternalInput")
    IDN = nc.dram_tensor("IDN", [128, 128], F32, kind="ExternalInput")
    KI = nc.dram_tensor("KI", [128, K], F32, kind="ExternalInput")
    MJ = nc.dram_tensor("MJ", [4, 128, NPG], F32, kind="ExternalInput")
    W1A = nc.dram_tensor("W1A", [4, 128, 16], F32, kind="ExternalInput")
    W1BR = nc.dram_tensor("W1BR", [1, 16], F32, kind="ExternalInput")
    B1R = nc.dram_tensor("B1R", [1, 16], F32, kind="ExternalInput")
    ONER = nc.dram_tensor("ONER", [1, 128], F32, kind="ExternalInput")
    W2T = nc.dram_tensor("W2T", [5, 16, 32], F32, kind="ExternalInput")
    B2 = nc.dram_tensor("B2", [32, 1], F32, kind="ExternalInput")
    D1R = nc.dram_tensor("D1R", [32, CONV2_LEN * 32], F32,
                         kind="ExternalInput")
    BD1 = nc.dram_tensor("BD1", [32, 1], F32, kind="ExternalInput")
    D2 = nc.dram_tensor("D2", [32, 2], F32, kind="ExternalInput")
    BD2 = nc.dram_tensor("BD2", [2, 1], F32, kind="ExternalInput")
    DIFF = nc.dram_tensor("DIFF", [2, 2], F32, kind="ExternalInput")
    if with_bias:
        ONE = nc.dram_tensor("ONE", [1, 128], F32, kind="ExternalInput")
        BCV = nc.dram_tensor("BCV", [NLAYERS, 1, 128], F32,
                             kind="ExternalInput")
        B5V = nc.dram_tensor("B5V", [1, 1], F32, kind="ExternalInput")
    OUT = nc.dram_tensor("OUT", [2, GPC], F32, kind="ExternalOutput")
    if debug:
        DBG_H = nc.dram_tensor("DBG_H", [NLAYERS, 128, NLOC], F32,
                               kind="ExternalOutput")
        DBG_H5 = nc.dram_tensor("DBG_H5", [GPC, 1, NPG], F32,
                                kind="ExternalOutput")
        DBG_RANK = nc.dram_tensor("DBG_RANK", [GPC, 128, 4], F32,
                                  kind="ExternalOutput")
        DBG_Y1 = nc.dram_tensor("DBG_Y1", [GPC, 128, 4 * 16], F32,
                                kind="ExternalOutput")
        DBG_SEL = nc.dram_tensor("DBG_SEL", [GPC, 16, K], F32,
                                 kind="ExternalOutput")
        DBG_Y2 = nc.dram_tensor("DBG_Y2", [32, GPC * CONV2_LEN], F32,
                                kind="ExternalOutput")

    TANH = mybir.ActivationFunctionType.Tanh
    RELU = mybir.ActivationFunctionType.Relu
    SIGM = mybir.ActivationFunctionType.Sigmoid
    ADD = mybir.AluOpType.add
    SUB = mybir.AluOpType.subtract
    MULT = mybir.AluOpType.mult
    MAX = mybir.AluOpType.max
    IS_GT = mybir.AluOpType.is_gt
    IS_EQ = mybir.AluOpType.is_equal

    with tile.TileContext(nc) as tc:
        with (
            tc.tile_pool(name="const", bufs=1) as cp,
            tc.tile_pool(name="hs", bufs=4) as hp,
            tc.tile_pool(name="at", bufs=1) as atp,
            tc.tile_pool(name="iv", bufs=1) as ivp,
            tc.tile_pool(name="hi", bufs=3) as hip,
            tc.tile_pool(name="lo", bufs=3) as lop,
            tc.tile_pool(name="sc5", bufs=3) as sc5p,
            tc.tile_pool(name="vbp", bufs=4) as vbp,
            tc.tile_pool(name="ptp", bufs=4) as ptp,
            tc.tile_pool(name="sm", bufs=6) as smp,
            tc.tile_pool(name="scr", bufs=3) as scp,
            tc.tile_pool(name="y1", bufs=4) as y1p_pool,
            tc.tile_pool(name="pslp", bufs=2, space="PSUM") as lpp,
            tc.tile_pool(name="ps512", bufs=2, space="PSUM") as ps5,
            tc.tile_pool(name="psrow", bufs=1, space="PSUM") as psr,
            tc.tile_pool(name="ps128", bufs=3, space="PSUM") as ps1,
        ):
            # ---- constant / weight loads (first-needed first) ----
            wc_sb = cp.tile([128, NLAYERS * 128], F32, tag="wc")
            h0 = hp.tile([128, NLOC], F32, tag="h")
            at_sb = []
            for g in range(GPC):
                at_t = atp.tile([128, 4 * NPG], DT_A, tag=f"at{g}",
                                name=f"at_t{g}")
                at_sb.append(at_t)

            def load_at(g):
                nc.sync.dma_start(
                    at_sb[g][:, :].rearrange("p (c n) -> p c n", c=4),
                    ATD[g, :, :, :].rearrange("c p n -> p c n"))

            invb = []
            for g in range(GPC):
                t = ivp.tile([128, NPG], F32, tag=f"invb{g}",
                             name=f"invb{g}")
                invb.append(t)
            inr = cp.tile([1, NLOC], F32, tag="inr")

            def load_inv(g):
                if g == 0:
                    nc.sync.dma_start(
                        inr[0:1, :].rearrange("q (g n) -> q g n", g=GPC),
                        INVR[:, :, :].rearrange("g q n -> q g n"))
                nc.gpsimd.partition_broadcast(
                    invb[g][:], inr[0:1, g * NPG:(g + 1) * NPG])

            # minimal first-matmul working set first: wc layer-0 slice, the
            # first h0 sliver, then graph-0's A chunks + inv; remaining
            # weights interleave behind
            nc.sync.dma_start(wc_sb[:, 0:128], WC[0, :, :])
            nc.sync.dma_start(h0[:, 0:512], H0T[:, 0:512])
            load_at(0)
            load_inv(0)
            nc.sync.dma_start(
                wc_sb[:, 128:512].rearrange("p (l c) -> p l c", l=3),
                WC[1:4, :, :].rearrange("l p c -> p l c"))
            for c in range(1, 4):
                nc.sync.dma_start(h0[:, c * 512:(c + 1) * 512],
                                  H0T[:, c * 512:(c + 1) * 512])
                load_at(c)
                load_inv(c)
            nc.sync.dma_start(h0[:, 2048:4096], H0T[:, 2048:4096])
            for g in range(4, GPC):
                load_at(g)
                load_inv(g)
            w5_sb = cp.tile([128, 1], F32, tag="w5")
            nc.sync.dma_start(w5_sb[:], W5[:])
            inv4_sb = cp.tile([128, 4 * GPC], F32, tag="inv4")
            nc.sync.dma_start(
                inv4_sb[:, :].rearrange("p (g c) -> p g c", g=GPC),
                INV4[:, :, :].rearrange("g p c -> p g c"))
            id_sb = cp.tile([128, 128], F32, tag="idn")
            nc.sync.dma_start(id_sb[:], IDN[:])
            ki_sb = cp.tile([128, K], F32, tag="ki")
            nc.sync.dma_start(ki_sb[:], KI[:])
            mj_sb = cp.tile([128, 4 * NPG], F32, tag="mj")
            nc.sync.dma_start(
                mj_sb[:, :].rearrange("p (c n) -> p c n", c=4),
                MJ[:, :, :].rearrange("c p n -> p c n"))
            w1a_sb = cp.tile([128, 64], F32, tag="w1a")
            nc.sync.dma_start(
                w1a_sb[:, :].rearrange("p (c n) -> p c n", c=4),
                W1A[:, :, :].rearrange("c p n -> p c n"))
            b1r_sb = cp.tile([1, 16], F32, tag="b1r")
            nc.sync.dma_start(b1r_sb[:], B1R[:])
            oner_sb = cp.tile([1, 128], F32, tag="oner")
            nc.sync.dma_start(oner_sb[:], ONER[:])
            w1br_sb = cp.tile([1, 16], F32, tag="w1br")
            nc.sync.dma_start(w1br_sb[:], W1BR[:])
            w1bb = cp.tile([128, 16], F32, tag="w1bb")
            nc.gpsimd.partition_broadcast(w1bb[:], w1br_sb[0:1, :])
            w2_sb = cp.tile([16, 160], F32, tag="w2t")
            nc.sync.dma_start(
                w2_sb[:, :].rearrange("p (t n) -> p t n", t=5),
                W2T[:, :, :].rearrange("t p n -> p t n"))
            b2_sb = cp.tile([32, 1], F32, tag="b2")
            nc.sync.dma_start(b2_sb[:], B2[:])
            d1_sb = cp.tile([32, CONV2_LEN * 32], F32, tag="d1r")
            nc.sync.dma_start(d1_sb[:], D1R[:])
            bd1_sb = cp.tile([32, 1], F32, tag="bd1")
            nc.sync.dma_start(bd1_sb[:], BD1[:])
            d2_sb = cp.tile([32, 2], F32, tag="d2")
            nc.sync.dma_start(d2_sb[:], D2[:])
            bd2_sb = cp.tile([2, 1], F32, tag="bd2")
            nc.sync.dma_start(bd2_sb[:], BD2[:])
            diff_sb = cp.tile([2, 2], F32, tag="diff")
            nc.sync.dma_start(diff_sb[:], DIFF[:])
            if with_bias:
                one_sb = cp.tile([1, 128], F32, tag="one")
                nc.sync.dma_start(one_sb[:], ONE[:])
                bcv_sb = []
                for l in range(NLAYERS):
                    t = cp.tile([1, 128], F32, tag=f"bcv{l}")
                    nc.sync.dma_start(t[:], BCV[l, :, :])
                    bcv_sb.append(t)
                b5v_sb = cp.tile([1, 1], F32, tag="b5v")
                nc.sync.dma_start(b5v_sb[:], B5V[:])
            y2all = cp.tile([32, GPC * CONV2_LEN], F32, tag="y2all")

            h_layers = []
            splits = {}

            def emit_lin(l, g, h_prev):
                """4 fp32 matmuls into one [128, 512] PSUM (node-chunk-major
                columns), then ONE hi/lo bf16 split (ACT + DVE)."""
                lp = lpp.tile([128, NPG], F32, tag="lp")
                for cc in range(4):
                    ch = 4 * g + cc
                    nc.tensor.matmul(
                        lp[:, cc * 128:(cc + 1) * 128],
                        h_prev[:, ch * 128:(ch + 1) * 128],
                        wc_sb[:, l * 128:(l + 1) * 128],
                        start=True, stop=not with_bias)
                    if with_bias:
                        nc.tensor.matmul(lp[:, cc * 128:(cc + 1) * 128],
                                         one_sb[:], bcv_sb[l][:],
                                         start=False, stop=True)
                if comp:
                    hi = hip.tile([128, NPG], BF16, tag="hi")
                    nc.scalar.copy(hi[:], lp[:])
                    lo = lop.tile([128, NPG], BF16, tag="lo")
                    nc.vector.tensor_tensor(out=lo[:], in0=lp[:],
                                            in1=hi[:], op=SUB)
                    splits[(l, g)] = (hi, lo)
                else:
                    lr = hip.tile([128, NPG], F32R, tag="hi")
                    nc.vector.tensor_copy(lr[:], lp[:])
                    splits[(l, g)] = (lr,)

            def emit_agg(l, g):
                """bf16 agg matmuls (exact integer A x hi/lo), inv row-scale
                (DVE), tanh (ACT) -> h feature-major."""
                arrs = splits.pop((l, g))
                sp = ps5.tile([128, NPG], F32, tag="ps512")
                nmm = 4 * len(arrs)
                i = 0
                for arr in arrs:
                    for cc in range(4):
                        nc.tensor.matmul(
                            sp[:], arr[:, cc * 128:(cc + 1) * 128],
                            at_sb[g][:, cc * NPG:(cc + 1) * NPG],
                            start=(i == 0), stop=(i == nmm - 1))
                        i += 1
                sc5 = sc5p.tile([128, NPG], F32, tag="sc5")
                nc.vector.tensor_tensor(out=sc5[:], in0=sp[:],
                                        in1=invb[g][:], op=MULT)
                nc.scalar.activation(
                    h_layers[l][:, g * NPG:(g + 1) * NPG], sc5[:], TANH)

            # ---- tail stages, software-pipelined across graphs ----
            vcols, vbs, ptts, y1ts, sels = {}, {}, {}, {}, {}

            def tail_t1(g):
                """h5 for graph g: fp32 matvecs for lin5, bf16-comp row-form
                aggregation (free=512 chains), inv + tanh on the row, then
                partition broadcast + node-major vcol via tiny transposes."""
                _mark(nc, f"t1_g{g}")
                h4 = h_layers[NLAYERS - 1]
                l5p = ps1.tile([128, 4], F32, tag="ps128")
                for cc in range(4):
                    ch = 4 * g + cc
                    nc.tensor.matmul(
                        l5p[:, cc:cc + 1],
                        h4[:, ch * 128:(ch + 1) * 128], w5_sb[:],
                        start=True, stop=not with_bias)
                    if with_bias:
                        nc.tensor.matmul(l5p[:, cc:cc + 1], one_sb[:],
                                         b5v_sb[:], start=False, stop=True)
                m5r = psr.tile([1, NPG], F32, tag="psrow")
                if comp:
                    hl5 = smp.tile([128, 8], BF16, tag="hl5")
                    nc.scalar.copy(hl5[:, 0:4], l5p[:])
                    nc.vector.tensor_tensor(out=hl5[:, 4:8], in0=l5p[:],
                                            in1=hl5[:, 0:4], op=SUB)
                    cols = [0, 1, 2, 3, 4, 5, 6, 7]
                else:
                    hl5 = smp.tile([128, 4], F32R, tag="hl5")
                    nc.vector.tensor_copy(hl5[:], l5p[:])
                    cols = [0, 1, 2, 3]
                for i, col in enumerate(cols):
                    sc = col % 4
                    nc.tensor.matmul(
                        m5r[:], hl5[:, col:col + 1],
                        at_sb[g][:, sc * NPG:(sc + 1) * NPG],
                        start=(i == 0), stop=(i == len(cols) - 1))
                m5v = smp.tile([1, NPG], F32, tag="m5v")
                nc.vector.tensor_tensor(out=m5v[:], in0=m5r[:],
                                        in1=invb[g][0:1, :], op=MULT)
                h5r = smp.tile([1, NPG], F32, tag="h5r")
                nc.scalar.activation(h5r[:], m5v[:], TANH)
                vb = vbp.tile([128, NPG], F32, tag="vb")
                nc.gpsimd.partition_broadcast(vb[:], h5r[0:1, :])
                # node-major vcol [128, 4] via 4 tiny PE transposes
                vpall = ps1.tile([128, 4], F32, tag="ps128")
                for cc in range(4):
                    nc.tensor.transpose(vpall[:, cc:cc + 1],
                                        h5r[0:1, cc * 128:(cc + 1) * 128],
                                        id_sb[0:1, 0:1])
                vcol = smp.tile([128, 4], F32, tag="vcol")
                nc.vector.tensor_copy(vcol[:], vpall[:])
                vcols[g] = vcol
                vbs[g] = vb
                if debug:
                    nc.sync.dma_start(DBG_H5[g, :, :], h5r[:])

            def tail_t2(g):
                """Exact stable ranks + one-hot selection matrix."""
                _mark(nc, f"t2_g{g}")
                vb, vcol = vbs[g], vcols[g]
                rank = smp.tile([128, 4], F32, tag="rank")
                for cc in range(4):
                    t1s = scp.tile([128, NPG], F32, tag="tt")
                    ra = smp.tile([128, 2], F32, tag="ra")
                    nc.vector.tensor_scalar(
                        out=t1s[:], in0=vb[:], scalar1=vcol[:, cc:cc + 1],
                        scalar2=None, op0=IS_GT, op1=ADD,
                        accum_out=ra[:, 0:1])
                    t2s = scp.tile([128, NPG], F32, tag="tt")
                    nc.vector.scalar_tensor_tensor(
                        out=t2s[:], in0=vb[:], scalar=vcol[:, cc:cc + 1],
                        in1=mj_sb[:, cc * NPG:(cc + 1) * NPG],
                        op0=IS_EQ, op1=MULT, accum_out=ra[:, 1:2])
                    nc.vector.tensor_tensor(
                        out=rank[:, cc:cc + 1], in0=ra[:, 0:1],
                        in1=ra[:, 1:2], op=ADD)
                if debug:
                    nc.sync.dma_start(DBG_RANK[g, :, :], rank[:])
                ptt = ptp.tile([128, 4 * K], F32, tag="pt")
                for cc in range(4):
                    nc.vector.tensor_scalar(
                        out=ptt[:, cc * K:(cc + 1) * K], in0=ki_sb[:],
                        scalar1=rank[:, cc:cc + 1], scalar2=None, op0=IS_EQ)
                ptts[g] = ptt

            def tail_t3(g):
                """conv1-first, node-major: y1T[d,o] = relu(xcat[:,d].W1 + b1)
                for all 512 nodes via fp32 matmuls with h chunks stationary
                (free=16), h5 rank-1 term via per-partition DVE scalar op,
                then one-hot row selection -> sel [16, K]. All exact fp32."""
                _mark(nc, f"t3_g{g}")
                vcol, ptt = vcols[g], ptts[g]
                y1t = y1p_pool.tile([128, 4 * 16], F32, tag="y1t")
                for cc in range(4):
                    ch = 4 * g + cc
                    p = ps1.tile([128, 16], F32, tag="ps128")
                    for l in range(NLAYERS):
                        nc.tensor.matmul(
                            p[:], h_layers[l][:, ch * 128:(ch + 1) * 128],
                            w1a_sb[:, l * 16:(l + 1) * 16],
                            start=(l == 0), stop=False)
                    nc.tensor.matmul(p[:], oner_sb[:], b1r_sb[:],
                                     start=False, stop=True)
                    # pre-relu y1 with the h5 rank-1 term, straight to SBUF
                    nc.vector.scalar_tensor_tensor(
                        out=y1t[:, cc * 16:(cc + 1) * 16], in0=w1bb[:],
                        scalar=vcol[:, cc:cc + 1],
                        in1=p[:], op0=MULT, op1=ADD)
                if debug:
                    nc.sync.dma_start(DBG_Y1[g, :, :], y1t[:])
                # select 64 rows (nodes) via one-hot matmuls; relu commutes
                # with one-hot selection so it's folded into the PSUM drain
                selp = ps1.tile([16, K], F32, tag="ps128")
                for cc in range(4):
                    nc.tensor.matmul(selp[:], y1t[:, cc * 16:(cc + 1) * 16],
                                     ptt[:, cc * K:(cc + 1) * K],
                                     start=(cc == 0), stop=(cc == 3))
                sel = smp.tile([16, K], F32, tag="sel")
                nc.scalar.activation(sel[:], selp[:], RELU)
                sels[g] = sel
                if debug:
                    nc.sync.dma_start(DBG_SEL[g, :, :], sel[:])

            def tail_t3b(g):
                """maxpool(2) -> conv2 -> relu into y2all."""
                _mark(nc, f"t3b_g{g}")
                sel = sels[g]
                mp = smp.tile([16, K // 2], F32, tag="mp")
                selv = sel[:].rearrange("p (a b) -> p a b", b=2)
                nc.vector.tensor_tensor(out=mp[:], in0=selv[:, :, 0:1],
                                        in1=selv[:, :, 1:2], op=MAX)
                y2p = ps1.tile([32, CONV2_LEN], F32, tag="ps128")
                for t5 in range(5):
                    nc.tensor.matmul(y2p[:], w2_sb[:, t5 * 32:(t5 + 1) * 32],
                                     mp[:, t5:t5 + CONV2_LEN],
                                     start=(t5 == 0), stop=(t5 == 4))
                nc.scalar.activation(
                    y2all[:, g * CONV2_LEN:(g + 1) * CONV2_LEN], y2p[:],
                    RELU, bias=b2_sb[:, 0:1])

            # ---- GCN layers 1..4, software-pipelined emission: the agg of
            # slot i-D is emitted after the lin of slot i so the PE stream
            # never head-of-line blocks on the hi/lo split. Tail stages for
            # graph g are emitted right after layer-4's agg(g), themselves
            # pipelined across graphs.
            _mark(nc, "layers")
            for l in range(NLAYERS):
                h_layers.append(hp.tile([128, NLOC], F32, tag="h",
                                        name=f"h{l + 1}"))

            def tail_after_agg(l, g):
                if l != NLAYERS - 1:
                    return
                tail_t1(g)
                if g >= 1:
                    tail_t2(g - 1)
                if g >= 2:
                    tail_t3(g - 2)
                if g >= 3:
                    tail_t3b(g - 3)

            slots = [(l, g) for l in range(NLAYERS) for g in range(GPC)]
            D = 1  # agg deferral distance (pipeline depth)
            for i, (l, g) in enumerate(slots):
                if g == 0:
                    _mark(nc, f"layer{l}")
                emit_lin(l, g, h0 if l == 0 else h_layers[l - 1])
                if i >= D:
                    la, ga = slots[i - D]
                    emit_agg(la, ga)
                    tail_after_agg(la, ga)
            for j in range(len(slots) - D, len(slots)):
                la, ga = slots[j]
                emit_agg(la, ga)
                tail_after_agg(la, ga)
            if debug:
                for l in range(NLAYERS):
                    nc.sync.dma_start(DBG_H[l, :, :], h_layers[l][:])
            tail_t2(GPC - 1)
            for g in range(GPC - 2, GPC):
                tail_t3(g)
            for g in range(GPC - 3, GPC):
                tail_t3b(g)
            if debug:
                nc.sync.dma_start(DBG_Y2[:], y2all[:])

            _mark(nc, "densetail")
            # ---- core-level dense tail (batched over the 8 graphs) ----
            h1p = ps1.tile([32, GPC], F32, tag="ps128")
            y2v = y2all[:].rearrange("p (g t) -> p g t", t=CONV2_LEN)
            for t5 in range(CONV2_LEN):
                nc.tensor.matmul(h1p[:], d1_sb[:, t5 * 32:(t5 + 1) * 32],
                                 y2v[:, :, t5:t5 + 1],
                                 start=(t5 == 0), stop=(t5 == CONV2_LEN - 1))
            h1s = smp.tile([32, GPC], F32, tag="h1s")
            nc.scalar.activation(h1s[:], h1p[:], RELU, bias=bd1_sb[:, 0:1])
            lgp = ps1.tile([2, GPC], F32, tag="ps128")
            nc.tensor.matmul(lgp[:], d2_sb[:], h1s[:], start=True, stop=True)
            lg = smp.tile([2, GPC], F32, tag="lg")
            nc.vector.tensor_scalar(out=lg[:], in0=lgp[:],
                                    scalar1=bd2_sb[:, 0:1], scalar2=None,
                                    op0=ADD)
            dfp = ps1.tile([2, GPC], F32, tag="ps128")
            nc.tensor.matmul(dfp[:], diff_sb[:], lg[:], start=True, stop=True)
            pr = smp.tile([2, GPC], F32, tag="pr")
            nc.scalar.activation(pr[:], dfp[:], SIGM)
            nc.sync.dma_start(OUT[:], pr[:])

    nc.compile()
    return nc


def _get_nc(dt_key, with_bias, debug):
    key = (dt_key, with_bias, debug)
    if key not in _NC_CACHE:
        mode = "fast" if dt_key == "f32r" else "comp"
        _NC_CACHE[key] = _build(mode, with_bias, debug)
    return _NC_CACHE[key]


def prepare_host(inputs, dt_key):
    """All host-side index preprocessing + per-core input maps."""
    comp = dt_key != "f32r"
    x = np.asarray(inputs["x"]).astype(np.int64)
    edge_index = np.asarray(inputs["edge_index"]).astype(np.int64)
    emb = np.ascontiguousarray(np.asarray(inputs["emb"], dtype=np.float32))
    W_convs = np.asarray(inputs["W_convs"], dtype=np.float32)
    b_convs = np.asarray(inputs["b_convs"], dtype=np.float32)
    W_last = np.asarray(inputs["W_last"], dtype=np.float32)
    b_last = np.asarray(inputs["b_last"], dtype=np.float32)
    conv1_w = np.asarray(inputs["conv1_w"], dtype=np.float32)
    conv1_b = np.asarray(inputs["conv1_b"], dtype=np.float32)
    conv2_w = np.asarray(inputs["conv2_w"], dtype=np.float32)
    conv2_b = np.asarray(inputs["conv2_b"], dtype=np.float32)
    d1_w = np.asarray(inputs["d1_w"], dtype=np.float32)
    d1_b = np.asarray(inputs["d1_b"], dtype=np.float32)
    d2_w = np.asarray(inputs["d2_w"], dtype=np.float32)
    d2_b = np.asarray(inputs["d2_b"], dtype=np.float32)

    src, dst = edge_index[0], edge_index[1]
    deg = (np.bincount(src, minlength=N_TOTAL) + 1).astype(np.float32)
    invdeg = (np.float32(1.0) / deg).astype(np.float32)
    gid = dst >> 9
    flat = (gid * NPG + (dst & 511)) * NPG + (src & 511)
    A = np.bincount(flat, minlength=NUM_GRAPHS * NPG * NPG)
    A = A.astype(np.float32).reshape(NUM_GRAPHS, NPG, NPG)
    idx = np.arange(NPG)
    A[:, idx, idx] += 1.0
    # A^T chunks: ATD[g, c, i, d] = A[g, d, c*128+i]
    AT = np.ascontiguousarray(A.transpose(0, 2, 1)).reshape(
        NUM_GRAPHS, 4, 128, NPG)
    if comp:
        AT = AT.astype(ml_dtypes.bfloat16)  # small ints: exact

    h0 = emb[x]  # [N, 128]
    with_bias = bool(np.any(b_convs) or np.any(b_last))

    w1 = np.ascontiguousarray(conv1_w[:, 0, :].T)  # [513, 16]
    shared = {
        "WC": np.ascontiguousarray(W_convs),
        "W5": np.ascontiguousarray(W_last),
        "IDN": np.eye(128, dtype=np.float32),
        "KI": np.ascontiguousarray(
            np.broadcast_to(np.arange(K, dtype=np.float32), (128, K))),
        "MJ": np.ascontiguousarray(
            (np.arange(NPG)[None, None, :]
             < (np.arange(4)[:, None, None] * 128
                + np.arange(128)[None, :, None])).astype(np.float32)),
        "W1A": np.ascontiguousarray(w1[:512].reshape(4, 128, 16)),
        "W1BR": np.ascontiguousarray(w1[512:513]),        # [1, 16]
        "B1R": np.ascontiguousarray(conv1_b.reshape(1, 16)),
        "ONER": np.ones((1, 128), dtype=np.float32),
        "W2T": np.ascontiguousarray(conv2_w.transpose(2, 1, 0)),
        "B2": np.ascontiguousarray(conv2_b.reshape(32, 1)),
        "D1R": np.ascontiguousarray(d1_w.reshape(DD, CONV2_LEN * 32)
                                    .astype(np.float32)),
        "BD1": np.ascontiguousarray(d1_b.reshape(32, 1)),
        "D2": np.ascontiguousarray(d2_w),
        "BD2": np.ascontiguousarray(d2_b.reshape(2, 1)),
        "DIFF": np.array([[1.0, -1.0], [-1.0, 1.0]], dtype=np.float32),
    }
    if with_bias:
        shared["ONE"] = np.ones((1, 128), dtype=np.float32)
        shared["BCV"] = np.ascontiguousarray(
            b_convs.reshape(NLAYERS, 1, 128))
        shared["B5V"] = np.ascontiguousarray(b_last.reshape(1, 1))

    invg = invdeg.reshape(NUM_GRAPHS, NPG)
    in_maps = []
    for c in range(NCORES):
        h0c = np.ascontiguousarray(h0[c * NLOC:(c + 1) * NLOC].T)
        iv = invg[c * GPC:(c + 1) * GPC]                  # [GPC, 512]
        m = dict(shared)
        m["H0T"] = h0c
        m["ATD"] = np.ascontiguousarray(AT[c * GPC:(c + 1) * GPC])
        m["INVR"] = np.ascontiguousarray(iv.reshape(GPC, 1, NPG))
        m["INV4"] = np.ascontiguousarray(
            iv.reshape(GPC, 4, 128).transpose(0, 2, 1))  # [GPC, 128, 4]
        in_maps.append(m)
    return in_maps, with_bias


def run(inputs, dt_key="f32", debug=False, **spmd_kwargs):
    in_maps, with_bias = prepare_host(inputs, dt_key)
    nc = _get_nc(dt_key, with_bias, debug)
    res = run_bass_kernel_spmd(nc, in_maps, core_ids=list(range(NCORES)),
                               **spmd_kwargs)
    out = np.empty((NUM_GRAPHS, 2), dtype=np.float32)
    for c in range(NCORES):
        out[c * GPC:(c + 1) * GPC, :] = res.results[c]["OUT"].T
    return out, res


def kernel(**inputs):
    out, _ = run(inputs, dt_key="f32")
    return out


# revision 36
# speedup vs baseline: 1.6268x; 1.0565x over previous
"""DGCNN (gnn_message_passing) Trainium2 Bass kernel, v2.

Strategy (data-parallel over graphs, 8 graphs per NeuronCore):
  - Host builds, per graph, the INTEGER operator A^T where A = adj-multiplicity
    + I (entries are small ints, exactly representable in bf16), shipped as
    4 chunks of [128, 512] bf16. The degree normalization inv = 1/deg is
    applied on-device AFTER the matmul (elementwise, exact fp32), so the
    aggregation matmul never rounds the operator.
  - Each GCN layer, per graph:
      lin  = h @ W          (fp32 matmuls, exact, node-major chunks)
      hi   = bf16(lin); lo = bf16(lin - hi)   (ACT copy + DVE subtract)
      u    = A^T-chunks x (hi | lo) -> PSUM [128f, 512d]  (8 bf16 matmuls at
             1 cyc/row -- 2x faster than one fp32 matmul, fp32-exact result)
      msg  = u * invb       (Pool elementwise)
      h'   = tanh(msg)      (ACT)
    This reproduces the reference to ~3e-7 (verified in numpy: zero top-k
    rank flips).
  - Layer 5 (h5): fp32/bf16-comp matvecs against the same A^T chunks,
    inv scale + tanh node-major, then transpose -> row form -> broadcast.
  - Ranks: exact stable rank[i] = #{v>v_i} + #{j<i: v==v_i} via DVE(is_gt)
    + Pool(is_eq*mask) passes; one-hot selection matrix PT from ranks.
  - Head (conv1-first): y1[o,d] = relu(W1 . xcat[:,d]) computed for ALL 512
    columns per graph via 4 f32r matmuls (free=512) + DVE rank-1 h5 update;
    then the CHEAP [16,512] tensor is transposed (4 tiny PE transposes) and
    64 columns selected by one-hot matmuls -- this replaces transposing all
    4 [128,512] h matrices per graph of the old design.
  - maxpool/conv2/dense head as small fp32 matmuls; final 2-class softmax
    via sigmoid of logit differences.

Modes: dt_key "f32" = compensated (default, ~3e-7 rel err);
       "f32r" = 1-term f32r aggregation (faster, ~1e-2 rel err).

Self-contained: hardcodes all shapes; no reads of /root/problem files.
"""

import sys

if "/opt/trn_rl_repo" not in sys.path:
    sys.path.insert(0, "/opt/trn_rl_repo")

import ml_dtypes
import numpy as np

import concourse.bacc as bacc
import concourse.mybir as mybir
import concourse.tile as tile
from concourse.bass_utils import run_bass_kernel_spmd

F32 = mybir.dt.float32
F32R = mybir.dt.float32r
BF16 = mybir.dt.bfloat16

NUM_GRAPHS = 64
NPG = 512  # nodes per graph
N_TOTAL = NUM_GRAPHS * NPG
EMB = 128
DIMF = 128
NLAYERS = 4
K = 64
NCORES = 8
GPC = NUM_GRAPHS // NCORES  # graphs per core = 8
NLOC = GPC * NPG  # local nodes = 4096
LATENT = NLAYERS * DIMF + 1  # 513
DD = (K - 2) // 2 + 1  # 32
CONV2_LEN = DD - 5 + 1  # 28

_NC_CACHE = {}
SECTION_MARKS = []  # (label, id_at_boundary) for profiling analysis


def _mark(nc, label):
    SECTION_MARKS.append((label, nc.next_id()))


def _build(mode, with_bias, debug):
    """Trace + compile the per-core Bass program (same on all 8 cores).

    mode: "comp" (bf16 A + hi/lo compensated agg, near-exact) or
          "fast" (f32r A + single rounded agg).
    """
    comp = mode == "comp"
    DT_A = BF16 if comp else F32R

    nc = bacc.Bacc("TRN2", target_bir_lowering=False, debug=False,
                   num_devices=NCORES)

    # ---- per-core DRAM I/O ----
    H0T = nc.dram_tensor("H0T", [128, NLOC], F32, kind="ExternalInput")
    ATD = nc.dram_tensor("ATD", [GPC, 4, 128, NPG], DT_A,
                         kind="ExternalInput")
    WC = nc.dram_tensor("WC", [NLAYERS, 128, 128], F32, kind="ExternalInput")
    W5 = nc.dram_tensor("W5", [128, 1], F32, kind="ExternalInput")
    INVR = nc.dram_tensor("INVR", [GPC, 1, NPG], F32, kind="ExternalInput")
    INV4 = nc.dram_tensor("INV4", [GPC, 128, 4], F32, kind="ExternalInput")
    IDN = nc.dram_tensor("IDN", [1, 1], F32, kind="ExternalInput")
    KI = nc.dram_tensor("KI", [128, K], F32, kind="ExternalInput")
    MJ = nc.dram_tensor("MJ", [4, 128, NPG], BF16, kind="ExternalInput")
    W1A = nc.dram_tensor("W1A", [4, 128, 16], F32, kind="ExternalInput")
    W1BR = nc.dram_tensor("W1BR", [1, 16], F32, kind="ExternalInput")
    B1R = nc.dram_tensor("B1R", [1, 16], F32, kind="ExternalInput")
    ONER = nc.dram_tensor("ONER", [1, 128], F32, kind="ExternalInput")
    W2T = nc.dram_tensor("W2T", [5, 16, 32], F32, kind="ExternalInput")
    B2 = nc.dram_tensor("B2", [32, 1], F32, kind="ExternalInput")
    D1R = nc.dram_tensor("D1R", [32, CONV2_LEN * 32], F32,
                         kind="ExternalInput")
    BD1 = nc.dram_tensor("BD1", [32, 1], F32, kind="ExternalInput")
    D2 = nc.dram_tensor("D2", [32, 2], F32, kind="ExternalInput")
    BD2 = nc.dram_tensor("BD2", [2, 1], F32, kind="ExternalInput")
    DIFF = nc.dram_tensor("DIFF", [2, 2], F32, kind="ExternalInput")
    if with_bias:
        ONE = nc.dram_tensor("ONE", [1, 128], F32, kind="ExternalInput")
        BCV = nc.dram_tensor("BCV", [NLAYERS, 1, 128], F32,
                             kind="ExternalInput")
        B5V = nc.dram_tensor("B5V", [1, 1], F32, kind="ExternalInput")
    OUT = nc.dram_tensor("OUT", [2, GPC], F32, kind="ExternalOutput")
    if debug:
        DBG_H = nc.dram_tensor("DBG_H", [NLAYERS, 128, NLOC], F32,
                               kind="ExternalOutput")
        DBG_H5 = nc.dram_tensor("DBG_H5", [GPC, 1, NPG], F32,
                                kind="ExternalOutput")
        DBG_RANK = nc.dram_tensor("DBG_RANK", [GPC, 128, 4], F32,
                                  kind="ExternalOutput")
        DBG_Y1 = nc.dram_tensor("DBG_Y1", [GPC, 128, 4 * 16], F32,
                                kind="ExternalOutput")
        DBG_SEL = nc.dram_tensor("DBG_SEL", [GPC, 16, K], F32,
                                 kind="ExternalOutput")
        DBG_Y2 = nc.dram_tensor("DBG_Y2", [32, GPC * CONV2_LEN], F32,
                                kind="ExternalOutput")

    TANH = mybir.ActivationFunctionType.Tanh
    RELU = mybir.ActivationFunctionType.Relu
    SIGM = mybir.ActivationFunctionType.Sigmoid
    ADD = mybir.AluOpType.add
    SUB = mybir.AluOpType.subtract
    MULT = mybir.AluOpType.mult
    MAX = mybir.AluOpType.max
    IS_GT = mybir.AluOpType.is_gt
    IS_EQ = mybir.AluOpType.is_equal

    with tile.TileContext(nc) as tc:
        with (
            tc.tile_pool(name="const", bufs=1) as cp,
            tc.tile_pool(name="hs", bufs=5) as hp,
            tc.tile_pool(name="at", bufs=1) as atp,
            tc.tile_pool(name="iv", bufs=1) as ivp,
            tc.tile_pool(name="hi", bufs=3) as hip,
            tc.tile_pool(name="lo", bufs=3) as lop,
            tc.tile_pool(name="sc5", bufs=3) as sc5p,
            tc.tile_pool(name="vbp", bufs=6) as vbp,
            tc.tile_pool(name="ptp", bufs=4) as ptp,
            tc.tile_pool(name="sm", bufs=6) as smp,
            tc.tile_pool(name="row", bufs=3) as rowp,
            tc.tile_pool(name="scr", bufs=2) as scp,
            tc.tile_pool(name="y1", bufs=4) as y1p_pool,
            tc.tile_pool(name="pslp", bufs=2, space="PSUM") as lpp,
            tc.tile_pool(name="ps512", bufs=2, space="PSUM") as ps5,
            tc.tile_pool(name="psrow", bufs=1, space="PSUM") as psr,
            tc.tile_pool(name="ps128", bufs=3, space="PSUM") as ps1,
        ):
            # ---- constant / weight loads (first-needed first) ----
            wc_sb = cp.tile([128, NLAYERS * 128], F32, tag="wc")
            h0 = hp.tile([128, NLOC], F32, tag="h")
            at_sb = []
            for g in range(GPC):
                at_t = atp.tile([128, 4 * NPG], DT_A, tag=f"at{g}",
                                name=f"at_t{g}")
                at_sb.append(at_t)

            def load_at(g):
                nc.sync.dma_start(
                    at_sb[g][:, :].rearrange("p (c n) -> p c n", c=4),
                    ATD[g, :, :, :].rearrange("c p n -> p c n"))

            invb = []
            for g in range(GPC):
                t = ivp.tile([128, NPG], F32, tag=f"invb{g}",
                             name=f"invb{g}")
                invb.append(t)
            def load_inv(g):
                ir = rowp.tile([1, NPG], F32, tag="inr")
                nc.sync.dma_start(ir[0:1, :], INVR[g, :, :])
                nc.gpsimd.partition_broadcast(invb[g][:], ir[0:1, :])

            # minimal first-matmul working set first: wc layer-0 slice, the
            # first h0 sliver, then graph-0's A chunks + inv; remaining
            # weights interleave behind
            nc.sync.dma_start(wc_sb[:, 0:128], WC[0, :, :])
            nc.sync.dma_start(h0[:, 0:512], H0T[:, 0:512])
            load_at(0)
            load_inv(0)
            nc.sync.dma_start(
                wc_sb[:, 128:512].rearrange("p (l c) -> p l c", l=3),
                WC[1:4, :, :].rearrange("l p c -> p l c"))
            for c in range(1, 4):
                nc.sync.dma_start(h0[:, c * 512:(c + 1) * 512],
                                  H0T[:, c * 512:(c + 1) * 512])
                load_at(c)
                load_inv(c)
            nc.sync.dma_start(h0[:, 2048:4096], H0T[:, 2048:4096])
            for g in range(4, GPC):
                load_at(g)
                load_inv(g)
            w5_sb = cp.tile([128, 1], F32, tag="w5")
            nc.sync.dma_start(w5_sb[:], W5[:])
            inv4_sb = cp.tile([128, 4 * GPC], F32, tag="inv4")
            nc.sync.dma_start(
                inv4_sb[:, :].rearrange("p (g c) -> p g c", g=GPC),
                INV4[:, :, :].rearrange("g p c -> p g c"))
            id_sb = cp.tile([1, 1], F32, tag="idn")
            nc.sync.dma_start(id_sb[:], IDN[:])
            ki_sb = cp.tile([128, K], F32, tag="ki")
            nc.sync.dma_start(ki_sb[:], KI[:])
            mj_sb = cp.tile([128, 4 * NPG], BF16, tag="mj")
            nc.sync.dma_start(
                mj_sb[:, :].rearrange("p (c n) -> p c n", c=4),
                MJ[:, :, :].rearrange("c p n -> p c n"))
            w1a_sb = cp.tile([128, 64], F32, tag="w1a")
            nc.sync.dma_start(
                w1a_sb[:, :].rearrange("p (c n) -> p c n", c=4),
                W1A[:, :, :].rearrange("c p n -> p c n"))
            b1r_sb = cp.tile([1, 16], F32, tag="b1r")
            nc.sync.dma_start(b1r_sb[:], B1R[:])
            oner_sb = cp.tile([1, 128], F32, tag="oner")
            nc.sync.dma_start(oner_sb[:], ONER[:])
            w1br_sb = cp.tile([1, 16], F32, tag="w1br")
            nc.sync.dma_start(w1br_sb[:], W1BR[:])
            w1bb = cp.tile([128, 16], F32, tag="w1bb")
            nc.gpsimd.partition_broadcast(w1bb[:], w1br_sb[0:1, :])
            w2_sb = cp.tile([16, 160], F32, tag="w2t")
            nc.sync.dma_start(
                w2_sb[:, :].rearrange("p (t n) -> p t n", t=5),
                W2T[:, :, :].rearrange("t p n -> p t n"))
            b2_sb = cp.tile([32, 1], F32, tag="b2")
            nc.sync.dma_start(b2_sb[:], B2[:])
            d1_sb = cp.tile([32, CONV2_LEN * 32], F32, tag="d1r")
            nc.sync.dma_start(d1_sb[:], D1R[:])
            bd1_sb = cp.tile([32, 1], F32, tag="bd1")
            nc.sync.dma_start(bd1_sb[:], BD1[:])
            d2_sb = cp.tile([32, 2], F32, tag="d2")
            nc.sync.dma_start(d2_sb[:], D2[:])
            bd2_sb = cp.tile([2, 1], F32, tag="bd2")
            nc.sync.dma_start(bd2_sb[:], BD2[:])
            diff_sb = cp.tile([2, 2], F32, tag="diff")
            nc.sync.dma_start(diff_sb[:], DIFF[:])
            if with_bias:
                one_sb = cp.tile([1, 128], F32, tag="one")
                nc.sync.dma_start(one_sb[:], ONE[:])
                bcv_sb = []
                for l in range(NLAYERS):
                    t = cp.tile([1, 128], F32, tag=f"bcv{l}")
                    nc.sync.dma_start(t[:], BCV[l, :, :])
                    bcv_sb.append(t)
                b5v_sb = cp.tile([1, 1], F32, tag="b5v")
                nc.sync.dma_start(b5v_sb[:], B5V[:])
            y2all = cp.tile([32, GPC * CONV2_LEN], F32, tag="y2all")

            h_layers = []
            splits = {}

            def emit_lin(l, g, h_prev):
                """4 fp32 matmuls into one [128, 512] PSUM (node-chunk-major
                columns), then ONE hi/lo bf16 split (ACT + DVE)."""
                lp = lpp.tile([128, NPG], F32, tag="lp")
                for cc in range(4):
                    ch = 4 * g + cc
                    nc.tensor.matmul(
                        lp[:, cc * 128:(cc + 1) * 128],
                        h_prev[:, ch * 128:(ch + 1) * 128],
                        wc_sb[:, l * 128:(l + 1) * 128],
                        start=True, stop=not with_bias)
                    if with_bias:
                        nc.tensor.matmul(lp[:, cc * 128:(cc + 1) * 128],
                                         one_sb[:], bcv_sb[l][:],
                                         start=False, stop=True)
                if comp:
                    hi = hip.tile([128, NPG], BF16, tag="hi")
                    nc.scalar.copy(hi[:], lp[:])
                    lo = lop.tile([128, NPG], BF16, tag="lo")
                    nc.vector.tensor_tensor(out=lo[:], in0=lp[:],
                                            in1=hi[:], op=SUB)
                    splits[(l, g)] = (hi, lo)
                else:
                    lr = hip.tile([128, NPG], F32R, tag="hi")
                    nc.vector.tensor_copy(lr[:], lp[:])
                    splits[(l, g)] = (lr,)

            def emit_agg(l, g):
                """bf16 agg matmuls (exact integer A x hi/lo), inv row-scale
                (DVE), tanh (ACT) -> h feature-major."""
                arrs = splits.pop((l, g))
                sp = ps5.tile([128, NPG], F32, tag="ps512")
                nmm = 4 * len(arrs)
                i = 0
                for arr in arrs:
                    for cc in range(4):
                        nc.tensor.matmul(
                            sp[:], arr[:, cc * 128:(cc + 1) * 128],
                            at_sb[g][:, cc * NPG:(cc + 1) * NPG],
                            start=(i == 0), stop=(i == nmm - 1))
                        i += 1
                sc5 = sc5p.tile([128, NPG], F32, tag="sc5")
                nc.vector.tensor_tensor(out=sc5[:], in0=sp[:],
                                        in1=invb[g][:], op=MULT)
                nc.scalar.activation(
                    h_layers[l][:, g * NPG:(g + 1) * NPG], sc5[:], TANH)

            # ---- tail stages, software-pipelined across graphs ----
            vcols, vbs, ptts, y1ts, sels = {}, {}, {}, {}, {}

            def tail_t1(g):
                """h5 for graph g: fp32 matvecs for lin5, bf16-comp row-form
                aggregation (free=512 chains), inv + tanh on the row, then
                partition broadcast + node-major vcol via tiny transposes."""
                _mark(nc, f"t1_g{g}")
                h4 = h_layers[NLAYERS - 1]
                l5p = ps1.tile([128, 4], F32, tag="ps128")
                for cc in range(4):
                    ch = 4 * g + cc
                    nc.tensor.matmul(
                        l5p[:, cc:cc + 1],
                        h4[:, ch * 128:(ch + 1) * 128], w5_sb[:],
                        start=True, stop=not with_bias)
                    if with_bias:
                        nc.tensor.matmul(l5p[:, cc:cc + 1], one_sb[:],
                                         b5v_sb[:], start=False, stop=True)
                m5r = psr.tile([1, NPG], F32, tag="psrow")
                if comp:
                    hl5 = smp.tile([128, 8], BF16, tag="hl5")
                    nc.scalar.copy(hl5[:, 0:4], l5p[:])
                    nc.vector.tensor_tensor(out=hl5[:, 4:8], in0=l5p[:],
                                            in1=hl5[:, 0:4], op=SUB)
                    cols = [0, 1, 2, 3, 4, 5, 6, 7]
                else:
                    hl5 = smp.tile([128, 4], F32R, tag="hl5")
                    nc.vector.tensor_copy(hl5[:], l5p[:])
                    cols = [0, 1, 2, 3]
                for i, col in enumerate(cols):
                    sc = col % 4
                    nc.tensor.matmul(
                        m5r[:], hl5[:, col:col + 1],
                        at_sb[g][:, sc * NPG:(sc + 1) * NPG],
                        start=(i == 0), stop=(i == len(cols) - 1))
                m5v = rowp.tile([1, NPG], F32, tag="m5v")
                nc.vector.tensor_tensor(out=m5v[:], in0=m5r[:],
                                        in1=invb[g][0:1, :], op=MULT)
                h5r = rowp.tile([1, NPG], F32, tag="h5r")
                nc.scalar.activation(h5r[:], m5v[:], TANH)
                vb = vbp.tile([128, NPG], F32, tag="vb")
                nc.gpsimd.partition_broadcast(vb[:], h5r[0:1, :])
                # node-major vcol [128, 4] via 4 tiny PE transposes
                vpall = ps1.tile([128, 4], F32, tag="ps128")
                for cc in range(4):
                    nc.tensor.transpose(vpall[:, cc:cc + 1],
                                        h5r[0:1, cc * 128:(cc + 1) * 128],
                                        id_sb[:])
                vcol = smp.tile([128, 4], F32, tag="vcol")
                nc.vector.tensor_copy(vcol[:], vpall[:])
                vcols[g] = vcol
                vbs[g] = vb
                if debug:
                    nc.sync.dma_start(DBG_H5[g, :, :], h5r[:])

            ranks = {}

            def t2_piece(g, cc):
                """Exact stable rank + one-hot column block for chunk cc."""
                if cc == 0:
                    _mark(nc, f"t2_g{g}")
                    ranks[g] = smp.tile([128, 4], F32, tag="rank",
                                        name=f"rank{g}")
                    ptts[g] = ptp.tile([128, 4 * K], F32, tag="pt",
                                       name=f"pt{g}")
                vb, vcol, rank, ptt = vbs[g], vcols[g], ranks[g], ptts[g]
                t1s = scp.tile([128, NPG], F32, tag="tt")
                ra = smp.tile([128, 2], F32, tag="ra")
                nc.vector.tensor_scalar(
                    out=t1s[:], in0=vb[:], scalar1=vcol[:, cc:cc + 1],
                    scalar2=None, op0=IS_GT, op1=ADD,
                    accum_out=ra[:, 0:1])
                t2s = scp.tile([128, NPG], F32, tag="tt")
                nc.vector.scalar_tensor_tensor(
                    out=t2s[:], in0=vb[:], scalar=vcol[:, cc:cc + 1],
                    in1=mj_sb[:, cc * NPG:(cc + 1) * NPG],
                    op0=IS_EQ, op1=MULT, accum_out=ra[:, 1:2])
                nc.vector.tensor_tensor(
                    out=rank[:, cc:cc + 1], in0=ra[:, 0:1],
                    in1=ra[:, 1:2], op=ADD)
                nc.vector.tensor_scalar(
                    out=ptt[:, cc * K:(cc + 1) * K], in0=ki_sb[:],
                    scalar1=rank[:, cc:cc + 1], scalar2=None, op0=IS_EQ)
                if debug and cc == 3:
                    nc.sync.dma_start(DBG_RANK[g, :, :], rank[:])

            y1ts = {}

            def t3_piece(g, cc):
                """conv1-first, node-major, one chunk: y1T[d,o] for 128 nodes
                via fp32 matmuls with h chunks stationary (free=16), h5
                rank-1 term via per-partition DVE scalar op. Exact fp32."""
                if cc == 0:
                    _mark(nc, f"t3_g{g}")
                    y1ts[g] = y1p_pool.tile([128, 4 * 16], F32, tag="y1t",
                                            name=f"y1t{g}")
                vcol, y1t = vcols[g], y1ts[g]
                ch = 4 * g + cc
                p = ps1.tile([128, 16], F32, tag="ps128")
                for l in range(NLAYERS):
                    nc.tensor.matmul(
                        p[:], h_layers[l][:, ch * 128:(ch + 1) * 128],
                        w1a_sb[:, l * 16:(l + 1) * 16],
                        start=(l == 0), stop=False)
                nc.tensor.matmul(p[:], oner_sb[:], b1r_sb[:],
                                 start=False, stop=True)
                # pre-relu y1 with the h5 rank-1 term, straight to SBUF
                nc.vector.scalar_tensor_tensor(
                    out=y1t[:, cc * 16:(cc + 1) * 16], in0=w1bb[:],
                    scalar=vcol[:, cc:cc + 1],
                    in1=p[:], op0=MULT, op1=ADD)

            def t3_sel(g):
                """Select 64 rows (nodes) via one-hot matmuls; relu commutes
                with one-hot selection so it's folded into the PSUM drain."""
                y1t, ptt = y1ts[g], ptts[g]
                if debug:
                    nc.sync.dma_start(DBG_Y1[g, :, :], y1t[:])
                selp = ps1.tile([16, K], F32, tag="ps128")
                for cc in range(4):
                    nc.tensor.matmul(selp[:], y1t[:, cc * 16:(cc + 1) * 16],
                                     ptt[:, cc * K:(cc + 1) * K],
                                     start=(cc == 0), stop=(cc == 3))
                sel = smp.tile([16, K], F32, tag="sel")
                nc.scalar.activation(sel[:], selp[:], RELU)
                sels[g] = sel
                if debug:
                    nc.sync.dma_start(DBG_SEL[g, :, :], sel[:])

            def tail_t3b(g):
                """maxpool(2) -> conv2 -> relu into y2all."""
                _mark(nc, f"t3b_g{g}")
                sel = sels[g]
                mp = smp.tile([16, K // 2], F32, tag="mp")
                selv = sel[:].rearrange("p (a b) -> p a b", b=2)
                nc.vector.tensor_tensor(out=mp[:], in0=selv[:, :, 0:1],
                                        in1=selv[:, :, 1:2], op=MAX)
                y2p = ps1.tile([32, CONV2_LEN], F32, tag="ps128")
                for t5 in range(5):
                    nc.tensor.matmul(y2p[:], w2_sb[:, t5 * 32:(t5 + 1) * 32],
                                     mp[:, t5:t5 + CONV2_LEN],
                                     start=(t5 == 0), stop=(t5 == 4))
                nc.scalar.activation(
                    y2all[:, g * CONV2_LEN:(g + 1) * CONV2_LEN], y2p[:],
                    RELU, bias=b2_sb[:, 0:1])

            # ---- GCN layers, group-major emission: graphs run in two
            # groups of 4 through all 4 layers, aggs deferred by D=4 slots
            # so the PE stream never head-of-line blocks on the hi/lo split.
            # Tail work is queued as small pieces and pumped into the
            # instruction streams between layer slots, so the first group's
            # DVE-heavy rank/select work overlaps the second group's layer
            # matmuls.
            _mark(nc, "layers")
            for l in range(NLAYERS):
                h_layers.append(hp.tile([128, NLOC], F32, tag="h",
                                        name=f"h{l + 1}"))

            tailq = []

            def queue_tail(g):
                for cc in range(4):
                    tailq.append(lambda g=g, cc=cc: t2_piece(g, cc))
                for cc in range(4):
                    tailq.append(lambda g=g, cc=cc: t3_piece(g, cc))
                tailq.append(lambda g=g: t3_sel(g))
                tailq.append(lambda g=g: tail_t3b(g))

            def pump(n):
                for _ in range(n):
                    if tailq:
                        tailq.pop(0)()

            def post_agg(l, g):
                if l == NLAYERS - 1:
                    tail_t1(g)
                    queue_tail(g)

            slots = [(l, grp * 4 + gg) for grp in range(2)
                     for l in range(NLAYERS) for gg in range(4)]
            D = 4  # agg deferral distance (pipeline depth)
            for i, (l, g) in enumerate(slots):
                if i >= D:
                    la, ga = slots[i - D]
                    emit_agg(la, ga)
                    post_agg(la, ga)
                if g % 4 == 0:
                    _mark(nc, f"layer{l}.{g // 4}")
                emit_lin(l, g, h0 if l == 0 else h_layers[l - 1])
                pump(3)
            for j in range(len(slots) - D, len(slots)):
                la, ga = slots[j]
                emit_agg(la, ga)
                post_agg(la, ga)
                pump(2)
            if debug:
                for l in range(NLAYERS):
                    nc.sync.dma_start(DBG_H[l, :, :], h_layers[l][:])
            _mark(nc, "drain")
            pump(len(tailq))
            if debug:
                nc.sync.dma_start(DBG_Y2[:], y2all[:])

            _mark(nc, "densetail")
            # ---- core-level dense tail (batched over the 8 graphs) ----
            h1p = ps1.tile([32, GPC], F32, tag="ps128")
            y2v = y2all[:].rearrange("p (g t) -> p g t", t=CONV2_LEN)
            for t5 in range(CONV2_LEN):
                nc.tensor.matmul(h1p[:], d1_sb[:, t5 * 32:(t5 + 1) * 32],
                                 y2v[:, :, t5:t5 + 1],
                                 start=(t5 == 0), stop=(t5 == CONV2_LEN - 1))
            h1s = smp.tile([32, GPC], F32, tag="h1s")
            nc.scalar.activation(h1s[:], h1p[:], RELU, bias=bd1_sb[:, 0:1])
            lgp = ps1.tile([2, GPC], F32, tag="ps128")
            nc.tensor.matmul(lgp[:], d2_sb[:], h1s[:], start=True, stop=True)
            lg = smp.tile([2, GPC], F32, tag="lg")
            nc.vector.tensor_scalar(out=lg[:], in0=lgp[:],
                                    scalar1=bd2_sb[:, 0:1], scalar2=None,
                                    op0=ADD)
            dfp = ps1.tile([2, GPC], F32, tag="ps128")
            nc.tensor.matmul(dfp[:], diff_sb[:], lg[:], start=True, stop=True)
            pr = smp.tile([2, GPC], F32, tag="pr")
            nc.scalar.activation(pr[:], dfp[:], SIGM)
            nc.sync.dma_start(OUT[:], pr[:])

    nc.compile()
    return nc


def _get_nc(dt_key, with_bias, debug):
    key = (dt_key, with_bias, debug)
    if key not in _NC_CACHE:
        mode = "fast" if dt_key == "f32r" else "comp"
        _NC_CACHE[key] = _build(mode, with_bias, debug)
    return _NC_CACHE[key]


def prepare_host(inputs, dt_key):
    """All host-side index preprocessing + per-core input maps."""
    comp = dt_key != "f32r"
    x = np.asarray(inputs["x"]).astype(np.int64)
    edge_index = np.asarray(inputs["edge_index"]).astype(np.int64)
    emb = np.ascontiguousarray(np.asarray(inputs["emb"], dtype=np.float32))
    W_convs = np.asarray(inputs["W_convs"], dtype=np.float32)
    b_convs = np.asarray(inputs["b_convs"], dtype=np.float32)
    W_last = np.asarray(inputs["W_last"], dtype=np.float32)
    b_last = np.asarray(inputs["b_last"], dtype=np.float32)
    conv1_w = np.asarray(inputs["conv1_w"], dtype=np.float32)
    conv1_b = np.asarray(inputs["conv1_b"], dtype=np.float32)
    conv2_w = np.asarray(inputs["conv2_w"], dtype=np.float32)
    conv2_b = np.asarray(inputs["conv2_b"], dtype=np.float32)
    d1_w = np.asarray(inputs["d1_w"], dtype=np.float32)
    d1_b = np.asarray(inputs["d1_b"], dtype=np.float32)
    d2_w = np.asarray(inputs["d2_w"], dtype=np.float32)
    d2_b = np.asarray(inputs["d2_b"], dtype=np.float32)

    src, dst = edge_index[0], edge_index[1]
    deg = (np.bincount(src, minlength=N_TOTAL) + 1).astype(np.float32)
    invdeg = (np.float32(1.0) / deg).astype(np.float32)
    gid = dst >> 9
    flat = (gid * NPG + (dst & 511)) * NPG + (src & 511)
    A = np.bincount(flat, minlength=NUM_GRAPHS * NPG * NPG)
    A = A.astype(np.float32).reshape(NUM_GRAPHS, NPG, NPG)
    idx = np.arange(NPG)
    A[:, idx, idx] += 1.0
    # A^T chunks: ATD[g, c, i, d] = A[g, d, c*128+i]
    AT = np.ascontiguousarray(A.transpose(0, 2, 1)).reshape(
        NUM_GRAPHS, 4, 128, NPG)
    if comp:
        AT = AT.astype(ml_dtypes.bfloat16)  # small ints: exact

    h0 = emb[x]  # [N, 128]
    with_bias = bool(np.any(b_convs) or np.any(b_last))

    w1 = np.ascontiguousarray(conv1_w[:, 0, :].T)  # [513, 16]
    shared = {
        "WC": np.ascontiguousarray(W_convs),
        "W5": np.ascontiguousarray(W_last),
        "IDN": np.ones((1, 1), dtype=np.float32),
        "KI": np.ascontiguousarray(
            np.broadcast_to(np.arange(K, dtype=np.float32), (128, K))),
        "MJ": np.ascontiguousarray(
            (np.arange(NPG)[None, None, :]
             < (np.arange(4)[:, None, None] * 128
                + np.arange(128)[None, :, None]))
            .astype(ml_dtypes.bfloat16)),
        "W1A": np.ascontiguousarray(w1[:512].reshape(4, 128, 16)),
        "W1BR": np.ascontiguousarray(w1[512:513]),        # [1, 16]
        "B1R": np.ascontiguousarray(conv1_b.reshape(1, 16)),
        "ONER": np.ones((1, 128), dtype=np.float32),
        "W2T": np.ascontiguousarray(conv2_w.transpose(2, 1, 0)),
        "B2": np.ascontiguousarray(conv2_b.reshape(32, 1)),
        "D1R": np.ascontiguousarray(d1_w.reshape(DD, CONV2_LEN * 32)
                                    .astype(np.float32)),
        "BD1": np.ascontiguousarray(d1_b.reshape(32, 1)),
        "D2": np.ascontiguousarray(d2_w),
        "BD2": np.ascontiguousarray(d2_b.reshape(2, 1)),
        "DIFF": np.array([[1.0, -1.0], [-1.0, 1.0]], dtype=np.float32),
    }
    if with_bias:
        shared["ONE"] = np.ones((1, 128), dtype=np.float32)
        shared["BCV"] = np.ascontiguousarray(
            b_convs.reshape(NLAYERS, 1, 128))
        shared["B5V"] = np.ascontiguousarray(b_last.reshape(1, 1))

    invg = invdeg.reshape(NUM_GRAPHS, NPG)
    in_maps = []
    for c in range(NCORES):
        h0c = np.ascontiguousarray(h0[c * NLOC:(c + 1) * NLOC].T)
        iv = invg[c * GPC:(c + 1) * GPC]                  # [GPC, 512]
        m = dict(shared)
        m["H0T"] = h0c
        m["ATD"] = np.ascontiguousarray(AT[c * GPC:(c + 1) * GPC])
        m["INVR"] = np.ascontiguousarray(iv.reshape(GPC, 1, NPG))
        m["INV4"] = np.ascontiguousarray(
            iv.reshape(GPC, 4, 128).transpose(0, 2, 1))  # [GPC, 128, 4]
        in_maps.append(m)
    return in_maps, with_bias


def run(inputs, dt_key="f32", debug=False, **spmd_kwargs):
    in_maps, with_bias = prepare_host(inputs, dt_key)
    nc = _get_nc(dt_key, with_bias, debug)
    res = run_bass_kernel_spmd(nc, in_maps, core_ids=list(range(NCORES)),
                               **spmd_kwargs)
    out = np.empty((NUM_GRAPHS, 2), dtype=np.float32)
    for c in range(NCORES):
        out[c * GPC:(c + 1) * GPC, :] = res.results[c]["OUT"].T
    return out, res


def kernel(**inputs):
    out, _ = run(inputs, dt_key="f32")
    return out
